# revision 63
# baseline (speedup 1.0000x reference)
"""DifferentialAttention Trainium2 kernel (8 NeuronCores, SPMD).

Sharding: data-parallel over batch B=4, tensor-parallel over heads
(2 cores per batch element, 8 heads each).  Each core computes the
partial projection output for its 8 heads; the host sums the two
bf16 partials per batch element in f32 and adds b_proj.

Per-core pipeline (bf16 matmuls, fp32 PSUM), 8 sweeps of
(head-pair hp, 512-col n-half nh):
  1. QKV^T = W_slice^T.T @ x^T            -> [channels, n] layout
  2. V via PE with swapped operands        -> [keys, ch|1] layout
  3. scores S^T[keys, n] per (head, half) with 4-way row groups;
     PSUM writes bank-aligned (tile_position requires it)
  4. exp on ACT (scale=1/8 folded), bf16 out; ACT runs ONLY exp -
     it is the binding engine (~133us); all evacuations are on DVE
  5. PV stationary-E into a single 3-bank tile: 16 chunks of
     [128 n, 65] (g = 4*ci + nsub) packed 7/7/2 per bank; col 64 of
     each chunk accumulates the softmax denominator
  6. combine on DVE: strided reciprocals + scalar_tensor_tensor
     (o1*r1 + o2*(-lam*r2)) -> oc[n, nsub, par, ch]
  7. oc -> oT[ch, n] via DMA transpose (XBAR) - no PE/PSUM cost
  8. proj = oT.T @ Wp: n-half 0 interleaved into the last sweep,
     n-half 1 pipelined per-nsub after the final combine
qkv/V/proj units share a 1-bank PSUM pool (disjoint in time).
"""

import sys

sys.path.insert(0, "/opt/trn_rl_repo")

import numpy as np
import ml_dtypes

B, N, C, H, HD = 4, 1024, 1024, 16, 64
LAMBDA_INIT = 0.8
BF16 = ml_dtypes.bfloat16

_PROG_CACHE = {}

LAG = 3
# combo ci=2g+i: 0=(even,h1) 1=(odd,h1) 2=(even,h2) 3=(odd,h2)
# score row group rg for ci (partition range of the half in qkvT)
RG = [0, 2, 1, 3]
# PV chunk g=4*ci+nsub -> (bank, 65-col slot): 7/7/2 packing
GB = [(g // 7, g % 7) for g in range(16)]
G_START = (0, 7, 14)  # first chunk written in each bank (zeroes it)
G_STOP = (6, 13, 15)  # last chunk written in each bank


def _build_program(debug=False):
    key = ("nc", debug)
    if key in _PROG_CACHE:
        return _PROG_CACHE[key]

    import concourse.mybir as mybir
    import concourse.tile as tile
    from concourse import bacc

    f32 = mybir.dt.float32
    b16 = mybir.dt.bfloat16
    Exp = mybir.ActivationFunctionType.Exp
    MUL = mybir.AluOpType.mult
    ADD = mybir.AluOpType.add

    nc = bacc.Bacc(None)

    # host layouts are partition-major so each DMA is one large transfer
    x_d = nc.dram_tensor("xT", [128, 8, N], b16, kind="ExternalInput")
    # wqkv columns reordered: block j'=2*hp+t (t=0 q, t=1 k), v at 1024:1536
    wqkv_d = nc.dram_tensor("wqkv", [128, 8, 1536], b16, kind="ExternalInput")
    wp_d = nc.dram_tensor("wp", [128, 4, C], b16, kind="ExternalInput")
    neglam_d = nc.dram_tensor("neglam", [128, 1], f32, kind="ExternalInput")
    ident_d = nc.dram_tensor("ident", [128, 128], b16, kind="ExternalInput")
    out_d = nc.dram_tensor("out", [8, 128, C], b16, kind="ExternalOutput")
    if debug:
        dbg_qkvT = nc.dram_tensor("dbg_qkvT", [128, 8, N], b16, kind="ExternalOutput")
        dbg_vsb = nc.dram_tensor("dbg_vsb", [128, 8, 8, 65], b16, kind="ExternalOutput")
        dbg_oc = nc.dram_tensor("dbg_oc", [128, 8, 4, 2, 64], b16, kind="ExternalOutput")
        dbg_oT = nc.dram_tensor("dbg_oT", [128, 4, N], b16, kind="ExternalOutput")

    with tile.TileContext(nc) as tc:
        with (
            tc.tile_pool(name="io", bufs=1) as iopool,
            tc.tile_pool(name="work", bufs=4) as wpool,
            tc.tile_pool(name="esb", bufs=12) as epool,
            tc.tile_pool(name="ocp", bufs=2) as ocpool,
            tc.tile_pool(name="pS", bufs=2, space="PSUM") as pS,
            tc.tile_pool(name="pO", bufs=1, space="PSUM") as pO,
            tc.tile_pool(name="pP", bufs=1, space="PSUM") as pP,
        ):
            xT = iopool.tile([128, 8, N], b16)
            wqkv = iopool.tile([128, 8, 1536], b16)
            wp = iopool.tile([128, 4, C], b16)
            neglam = iopool.tile([128, 1], f32)
            # qkvT chunk j=2*hp+t: partitions 0-63 even head d0..63,
            # 64-127 odd head d0..63
            qkvT = iopool.tile([128, 8, N], b16)
            # V in [keys, channels] layout; col 64 of each head = ones
            vsb = iopool.tile([128, 8, 8, 65], b16)
            # transposed attention out for proj: [ch-part, hp, n]
            oT = iopool.tile([128, 4, N], b16)
            ident = iopool.tile([128, 128], b16)
            if debug:
                dbg_oc_t = iopool.tile([128, 8, 4, 2, 64], b16)

            warm = iopool.tile([128, 256], b16)
            nc.gpsimd.memset(warm[:], 0.5)
            nc.gpsimd.memset(vsb[:, :, :, 64:65], 1.0)

            # ---------------- DMA in (strict need order) ----------------
            # The shared DMA slot grants waiting transfers in ARBITRARY
            # order, so a late-need transfer that queues early can starve a
            # critical one.  Put everything on the sync queue in exact need
            # order - its ~1.3us per-issue pacing self-throttles - except
            # w[j0,j1] which rides the otherwise-empty scalar queue.
            nc.scalar.dma_start(wqkv[:, :, 0:256], wqkv_d[:, :, 0:256])
            for cc2 in range(4):
                nc.sync.dma_start(
                    xT[:, 2 * cc2 : 2 * cc2 + 2, 0:512],
                    x_d[:, 2 * cc2 : 2 * cc2 + 2, 0:512],
                )
            # v weights before xh1: V-u0 consumes them first (it only needs
            # xh0), while xh1's first consumer is the k2 unit a bit later
            nc.sync.dma_start(wqkv[:, :, 1024:1280], wqkv_d[:, :, 1024:1280])
            nc.sync.dma_start(wqkv[:, :, 1280:1536], wqkv_d[:, :, 1280:1536])
            for h4 in range(2):
                nc.sync.dma_start(
                    xT[:, 4 * h4 : 4 * h4 + 4, 512:1024],
                    x_d[:, 4 * h4 : 4 * h4 + 4, 512:1024],
                )
            nc.gpsimd.dma_start(neglam[:], neglam_d[:])
            for hp in range(1, 4):
                c0 = hp * 256
                nc.sync.dma_start(wqkv[:, :, c0 : c0 + 256], wqkv_d[:, :, c0 : c0 + 256])
            nc.sync.dma_start(wp[:], wp_d[:])
            nc.sync.dma_start(ident[:], ident_d[:])

            # PE warm-up during the input DMAs: the pstate model needs ~3us
            # of continuous execution to reach full clock, so burn it on
            # dummy matmuls into a single scratch PSUM tile (one slot alloc
            # so the pP ring is not serialized)
            wps = pP.tile([128, 512], f32, tag="p", name="warm")
            for w in range(13):
                nc.tensor.matmul(
                    wps[0:1, 0:256], warm[:, 0:1], warm[:], start=True, stop=True
                )

            # ---------------- filler units ------------------------------
            def emit_qkv_unit(hp, t, nh, pool=None):
                # one [128,512] q/k projection chunk -> qkvT[:, 2hp+t, nh]
                j = 2 * hp + t
                pool = pool or pP
                ps = pool.tile([128, 512], f32, tag=pool.name[1].lower(),
                               name=f"qkv{j}_{nh}")
                for cc in range(8):
                    nc.tensor.matmul(
                        ps[:],
                        wqkv[:, cc, j * 128 : (j + 1) * 128],
                        xT[:, cc, nh * 512 : (nh + 1) * 512],
                        start=(cc == 0),
                        stop=(cc == 7),
                    )
                nc.vector.tensor_copy(qkvT[:, j, nh * 512 : (nh + 1) * 512], ps[:])

            vparts = {}

            def emit_v_half(mc, part):
                # v -> [keys, channels] layout (operands swapped); DVE evac;
                # emitted in two half-contraction quanta to smooth PE load
                if part == 0:
                    vparts[mc] = pP.tile([128, 512], f32, tag="p", name=f"v{mc}")
                ps = vparts[mc]
                for cc in range(4 * part, 4 * part + 4):
                    nc.tensor.matmul(
                        ps[:],
                        xT[:, cc, mc * 128 : (mc + 1) * 128],
                        wqkv[:, cc, 1024:1536],
                        start=(cc == 0),
                        stop=(cc == 7),
                    )
                if part == 1:
                    nc.vector.tensor_copy(
                        vsb[:, mc, :, 0:64], ps.rearrange("p (g d) -> p g d", g=8)
                    )

            def emit_v_unit(mc):
                emit_v_half(mc, 0)
                emit_v_half(mc, 1)

            def emit_proj_unit(ncc, jh, pool, on_act=False):
                # out[ncc n-chunk, jh 512 out-ch] = oT.T @ wp, K=512 (4 ci)
                # tail units evacuate on ACT (idle once the exps are done)
                ps = pool.tile([128, 512], f32, tag=pool.name[1].lower(),
                               name=f"proj{ncc}_{jh}")
                for ci in range(4):
                    nc.tensor.matmul(
                        ps[:],
                        oT[:, ci, ncc * 128 : (ncc + 1) * 128],
                        wp[:, ci, jh * 512 : (jh + 1) * 512],
                        start=(ci == 0),
                        stop=(ci == 3),
                    )
                osb = wpool.tile([128, 512], b16, tag="osb", bufs=6)
                if on_act:
                    # tail: ACT evac + scalar-queue DMA keeps the sync queue
                    # free for the final transposes' waits
                    nc.scalar.copy(osb[:], ps[:])
                    nc.scalar.dma_start(out_d[ncc, :, jh * 512 : (jh + 1) * 512], osb[:])
                else:
                    nc.vector.tensor_copy(osb[:], ps[:])
                    nc.sync.dma_start(out_d[ncc, :, jh * 512 : (jh + 1) * 512], osb[:])

            projA = {}

            def emit_proj_partial(ncc, jh):
                # n-half-1 proj split: accumulate head-pairs 0-2 into PSUM
                # during sweep 6-7 slack, park the partial in SBUF f32
                ps = pP.tile([128, 512], f32, tag="p", name=f"pp{ncc}_{jh}")
                for ci in range(3):
                    nc.tensor.matmul(
                        ps[:],
                        oT[:, ci, ncc * 128 : (ncc + 1) * 128],
                        wp[:, ci, jh * 512 : (jh + 1) * 512],
                        start=(ci == 0),
                        stop=(ci == 2),
                    )
                pa = wpool.tile([128, 512], f32, tag=f"pa{ncc}_{jh}", bufs=1)
                nc.vector.tensor_copy(pa[:], ps[:])
                projA[(ncc, jh)] = pa

            def emit_proj_final(ncc, jh, pool):
                # tail: one head-pair-3 matmul + DVE add of the parked partial
                ps = pool.tile([128, 512], f32, tag=pool.name[1].lower(),
                               name=f"pf{ncc}_{jh}")
                nc.tensor.matmul(
                    ps[:],
                    oT[:, 3, ncc * 128 : (ncc + 1) * 128],
                    wp[:, 3, jh * 512 : (jh + 1) * 512],
                    start=True,
                    stop=True,
                )
                osb = wpool.tile([128, 512], b16, tag="osb", bufs=6)
                nc.vector.tensor_tensor(osb[:], ps[:], projA[(ncc, jh)][:], ADD)
                nc.sync.dma_start(out_d[ncc, :, jh * 512 : (jh + 1) * 512], osb[:])

            # ---------------- attention pieces --------------------------
            def emit_scores_exp(hp, nh, mc):
                # 4 score matmuls + 2 exps; returns e tiles [g0, g1]
                cur = []
                for g in range(2):
                    s_ps = pS.tile([128, 2, 512], f32, tag="s")
                    for i in range(2):
                        rg = RG[2 * g + i]
                        nc.tensor.matmul(
                            s_ps[:, i, :],
                            qkvT[
                                32 * rg : 32 * rg + 32,
                                2 * hp + 1,
                                mc * 128 : (mc + 1) * 128,
                            ],
                            qkvT[
                                32 * rg : 32 * rg + 32,
                                2 * hp,
                                nh * 512 : (nh + 1) * 512,
                            ],
                            start=True,
                            stop=True,
                            tile_position=(32 * rg, 0),
                        )
                    e_sb = epool.tile([128, 2, 512], b16, tag="e")
                    nc.scalar.activation(e_sb[:], s_ps[:], Exp, scale=0.125)
                    cur.append(e_sb)
                return cur

            def emit_pv(hp, etiles, o, mc):
                # stationary-E PV into the 3-bank packed tile o [128,3,512]
                # chunk g=4*ci+nsub at (bank g//7, col (g%7)*65); start=True
                # zeroes the whole bank so only its first chunk may set it
                for ci in range(4):
                    par = ci % 2
                    for nsub in range(4):
                        g = 4 * ci + nsub
                        b, sl = GB[g]
                        nc.tensor.matmul(
                            o[:, b, sl * 65 : sl * 65 + 65],
                            etiles[ci // 2][:, par, nsub * 128 : (nsub + 1) * 128],
                            vsb[:, mc, 2 * hp + par, :],
                            start=(mc == 0 and g in G_START),
                            stop=(mc == 7 and g in G_STOP),
                            skip_group_check=True,
                        )

            def emit_oraw(o):
                # bulk-evacuate the packed accumulator to SBUF (3 copies,
                # one per bank) so the PSUM tile is released fast - the
                # normalization then runs off the critical path from SBUF
                oraw = wpool.tile([128, 16, 65], f32, tag="oraw")
                orv = oraw.rearrange("p g c -> p (g c)")
                for b, cnt in ((0, 7), (1, 7), (2, 2)):
                    nc.vector.tensor_copy(
                        orv[:, b * 455 : b * 455 + cnt * 65], o[:, b, 0 : cnt * 65]
                    )
                return oraw

            def emit_recips(oraw):
                # per-partition reciprocals of the stride-65 denominators
                r = wpool.tile([128, 16, 1], f32, tag="r")
                nc.vector.reciprocal(r[:], oraw[:, :, 64:65])
                # -lam fold on the h2 chunks (g 8..15)
                nc.vector.tensor_scalar_mul(r[:, 8:16], r[:, 8:16], neglam[:])
                return r

            def emit_combine_chunk(hp, oraw, r, oc, nsub):
                # one nsub (128 n cols): 2 parities, fused on DVE, all SBUF
                for par in range(2):
                    g1 = 4 * par + nsub
                    g2 = 8 + 4 * par + nsub
                    t = wpool.tile([128, 64], f32, tag=f"t{par}")
                    nc.vector.tensor_scalar_mul(
                        t[:], oraw[:, g2, 0:64], r[:, g2]
                    )
                    nc.vector.scalar_tensor_tensor(
                        oc[:, nsub, par, :],
                        oraw[:, g1, 0:64],
                        r[:, g1],
                        t[:],
                        MUL,
                        ADD,
                    )

            def emit_combine(hp, nh, o):
                oraw = emit_oraw(o)
                r = emit_recips(oraw)
                oc = ocpool.tile([128, 4, 2, 64], b16, tag="oc")
                for nsub in range(4):
                    emit_combine_chunk(hp, oraw, r, oc, nsub)
                if debug:
                    nc.vector.tensor_copy(dbg_oc_t[:, 2 * hp + nh], oc[:])
                return oc

            def emit_transpose(hp, nh, oc, nsub):
                # oc[128 n, par, 64ch] -> oT[128 ch, n] via DMA xbar
                n0 = nh * 512 + nsub * 128
                nc.sync.dma_start_transpose(oT[:, hp, n0 : n0 + 128], oc[:, nsub])

            def emit_transpose_pe(hp, nh, oc, nsub):
                # final-sweep transpose on PE (PSUM is free, and the ~2.4us
                # DMA-transpose latency would sit on the critical tail)
                n0 = nh * 512 + nsub * 128
                trb = pS.tile([128, 128], b16, tag="s", name="trb")
                nc.tensor.matmul(trb[:], oc[:, nsub], ident[:], is_transpose=True)
                nc.scalar.copy(oT[:, hp, n0 : n0 + 128], trb[:])

            # ---------------- filler schedule ---------------------------
            filler = {}

            def add(slot, fn, *args):
                filler.setdefault(slot, []).append((fn, args))

            # sweep 0 (hp0,nh0): V units + rest of hp0 qkv.  V-u(k) must be
            # emitted by the fillers of slot k+LAG (its PV consumer); the
            # second-half hp1 units are deferred to sweep 2's slack.
            add((0, 0), emit_v_unit, 0)
            add((0, 1), emit_v_unit, 1)
            add((0, 2), emit_qkv_unit, 0, 1, 1)  # k hp0 keys 512-1023 (mc4+)
            add((0, 3), emit_v_unit, 2)
            add((0, 4), emit_v_unit, 3)
            add((0, 5), emit_qkv_unit, 0, 0, 1)  # q hp0 n 512+ (sweep 1)
            add((0, 6), emit_v_unit, 4)
            add((0, 7), emit_v_unit, 5)
            add((1, 0), emit_v_unit, 6)
            add((1, 0), emit_v_unit, 7)
            # sweep 1: first-half hp1 qkv (needed by sweep 2 start)
            add((1, 2), emit_qkv_unit, 1, 1, 0)
            add((1, 5), emit_qkv_unit, 1, 0, 0)
            # sweep 2: rest of hp1 (k-u2 by slot 4, q-u2 by sweep 3)
            add((2, 0), emit_qkv_unit, 1, 1, 1)
            add((2, 2), emit_qkv_unit, 1, 0, 1)
            # sweeps 2-3: hp2 (needed by sweep 4)
            add((2, 5), emit_qkv_unit, 2, 1, 0)
            add((3, 0), emit_qkv_unit, 2, 1, 1)
            add((3, 3), emit_qkv_unit, 2, 0, 0)
            add((3, 6), emit_qkv_unit, 2, 0, 1)
            # sweeps 4-5: hp3 (needed by sweep 6)
            add((4, 2), emit_qkv_unit, 3, 1, 0)
            add((4, 6), emit_qkv_unit, 3, 1, 1)
            add((5, 2), emit_qkv_unit, 3, 0, 0)
            add((5, 6), emit_qkv_unit, 3, 0, 1)
            # sweep 7: proj units for n-half 0 ride the per-slot PE slack
            # (evacs on DVE so they don't steal ACT from the exps).  They
            # must come at mc>=2: sweep 6's transposes land in the mc==1
            # combine block, and a unit emitted before them would read
            # oT[:, 3, :] with no registered writer.
            add((7, 2), emit_proj_unit, 0, 0, pP)
            add((7, 3), emit_proj_unit, 0, 1, pP)
            add((7, 4), emit_proj_unit, 1, 0, pP)
            add((7, 5), emit_proj_unit, 1, 1, pP)
            add((7, 6), emit_proj_unit, 2, 0, pP)
            add((7, 7), emit_proj_unit, 2, 1, pP)

            # qkv for the first sweep must precede it.  k and q interleave
            # per 2-cc chunk (k on pP, q on a free pS slot) so both chase
            # the xh0 chunk arrivals; k keys 0-127 evacuate first since the
            # first score matmul only needs that slice.
            kps = pP.tile([128, 512], f32, tag="p", name="k0pre")
            qps = pS.tile([128, 512], f32, tag="s", name="q0pre")
            for cc in range(8):
                nc.tensor.matmul(
                    kps[:], wqkv[:, cc, 128:256], xT[:, cc, 0:512],
                    start=(cc == 0), stop=(cc == 7),
                )
                nc.tensor.matmul(
                    qps[:], wqkv[:, cc, 0:128], xT[:, cc, 0:512],
                    start=(cc == 0), stop=(cc == 7),
                )
            nc.vector.tensor_copy(qkvT[:, 1, 0:128], kps[:, 0:128])
            nc.vector.tensor_copy(qkvT[:, 0, 0:512], qps[:])
            nc.vector.tensor_copy(qkvT[:, 1, 128:512], kps[:, 128:512])

            # ---------------- main pipeline -----------------------------
            # software-pipelined across sweep boundaries: the tail PVs of
            # sweep s are emitted after sweep s+1's first scores/exp so the
            # next exp is never behind them on PE; combine lands at mc==1
            pending = None  # (hp, nh, o_tile)
            tailpv = None  # (hp, edeque, o_tile)
            for s in range(8):
                hp, nh = s // 2, s % 2
                o = None
                edeque = []
                for mc in range(8):
                    edeque.append(emit_scores_exp(hp, nh, mc))
                    for fn, args in filler.get((s, mc), ()):
                        fn(*args)
                    if mc == 0 and tailpv is not None:
                        for mcp in range(8 - LAG, 8):
                            emit_pv(tailpv[0], tailpv[1][mcp], tailpv[2], mcp)
                        tailpv = None
                    if mc == 1 and pending is not None:
                        oc = emit_combine(*pending)
                        for nsub in range(4):
                            emit_transpose(pending[0], pending[1], oc, nsub)
                        pending = None
                    if mc >= LAG:
                        if o is None:
                            o = pO.tile([128, 3, 512], f32, tag="o", name=f"o{s}")
                        emit_pv(hp, edeque[mc - LAG], o, mc - LAG)
                pending = (hp, nh, o)
                tailpv = (hp, edeque, o)

            # final sweep: tail PVs, then chunked combine -> transpose ->
            # proj pipeline; two held-back n-half-0 proj units keep PE warm
            hp, nh, o = pending
            for mcp in range(8 - LAG, 8):
                emit_pv(hp, tailpv[1][mcp], o, mcp)
            oraw = emit_oraw(o)
            r = emit_recips(oraw)
            # held-back n-half-0 proj units fill the combine window on PE;
            # evacs stay on DVE so ACT only carries the tr copies and the
            # two engines pipeline the tail instead of serializing on ACT
            emit_proj_unit(3, 0, pP)
            emit_proj_unit(3, 1, pS)
            oc = ocpool.tile([128, 4, 2, 64], b16, tag="oc")
            for nsub in range(4):
                emit_combine_chunk(hp, oraw, r, oc, nsub)
                emit_transpose_pe(hp, nh, oc, nsub)
                emit_proj_unit(4 + nsub, 0, pP if nsub % 2 else pS)
                # last group: second evac on the now-idle ACT engine so the
                # two final evac+DMA chains run on parallel engines
                emit_proj_unit(4 + nsub, 1, pS if nsub % 2 else pP,
                               on_act=(nsub == 3))
            if debug:
                nc.vector.tensor_copy(dbg_oc_t[:, 2 * hp + nh], oc[:])

            if debug:
                nc.sync.dma_start(dbg_qkvT[:], qkvT[:])
                nc.sync.dma_start(dbg_vsb[:], vsb[:])
                nc.sync.dma_start(dbg_oc[:], dbg_oc_t[:])
                nc.sync.dma_start(dbg_oT[:], oT[:])

    nc.compile()
    _PROG_CACHE[key] = nc
    return nc


def _prep_core_inputs(x, W_qkv, W_proj, neg_lam):
    """Host-side shard prep. Returns in_maps for the 8 cores."""
    W4 = np.asarray(W_qkv, np.float32).reshape(3, H, HD, C)
    in_maps = []
    for core in range(8):
        b, hg = divmod(core, 2)
        xT = (
            np.ascontiguousarray(np.asarray(x[b], np.float32).T)
            .reshape(8, 128, N)
            .transpose(1, 0, 2)
            .astype(BF16)
        )
        wsl = W4[:, hg * 8 : (hg + 1) * 8]  # [3, 8 heads, 64, 1024]
        # columns [t(3), head(8), d(64)]; reorder q/k to j'=2*hp+t blocks
        Wcols = np.ascontiguousarray(wsl.transpose(3, 0, 1, 2).reshape(C, 1536))
        Wnew = np.empty_like(Wcols)
        for hp in range(4):
            for t in range(2):
                src = t * 512 + hp * 128
                dst = (2 * hp + t) * 128
                Wnew[:, dst : dst + 128] = Wcols[:, src : src + 128]
        Wnew[:, 1024:1536] = Wcols[:, 1024:1536]
        wqkv = Wnew.reshape(8, 128, 1536).transpose(1, 0, 2).astype(BF16)
        wp = (
            np.ascontiguousarray(
                np.asarray(W_proj, np.float32)[:, hg * 512 : (hg + 1) * 512].T
            )
            .reshape(4, 128, C)
            .transpose(1, 0, 2)
            .astype(BF16)
        )
        in_maps.append(
            {
                "xT": np.ascontiguousarray(xT),
                "wqkv": np.ascontiguousarray(wqkv),
                "wp": np.ascontiguousarray(wp),
                "neglam": np.full((128, 1), neg_lam, np.float32),
                "ident": np.eye(128, dtype=np.float32).astype(BF16),
            }
        )
    return in_maps


def kernel(x, W_qkv, W_proj, b_proj, lambda_q1, lambda_k1, lambda_q2, lambda_k2):
    from concourse.bass_utils import run_bass_kernel_spmd

    lq1 = np.asarray(lambda_q1, np.float64)
    lk1 = np.asarray(lambda_k1, np.float64)
    lq2 = np.asarray(lambda_q2, np.float64)
    lk2 = np.asarray(lambda_k2, np.float64)
    lam = float(np.mean(np.exp(lq1 * lk1) - np.exp(lq2 * lk2) + LAMBDA_INIT))

    nc = _build_program()
    in_maps = _prep_core_inputs(x, W_qkv, W_proj, -lam)
    res = run_bass_kernel_spmd(nc, in_maps, core_ids=list(range(8)))
    _PROG_CACHE["last_result"] = res

    bp = np.asarray(b_proj, np.float32)
    out = np.empty((B, N, C), np.float32)
    for b in range(B):
        p0 = res.results[2 * b]["out"].astype(np.float32).reshape(N, C)
        p1 = res.results[2 * b + 1]["out"].astype(np.float32).reshape(N, C)
        out[b] = p0 + p1 + bp[None, :]
    return out


# revision 64
# speedup vs baseline: 1.0013x; 1.0013x over previous
"""DifferentialAttention Trainium2 kernel (8 NeuronCores, SPMD).

Sharding: data-parallel over batch B=4, tensor-parallel over heads
(2 cores per batch element, 8 heads each).  Each core computes the
partial projection output for its 8 heads; the host sums the two
bf16 partials per batch element in f32 and adds b_proj.

Per-core pipeline (bf16 matmuls, fp32 PSUM), 8 sweeps of
(head-pair hp, 512-col n-half nh):
  1. QKV^T = W_slice^T.T @ x^T            -> [channels, n] layout
  2. V via PE with swapped operands        -> [keys, ch|1] layout
  3. scores S^T[keys, n] per (head, half) with 4-way row groups;
     PSUM writes bank-aligned (tile_position requires it)
  4. exp on ACT (scale=1/8 folded), bf16 out; ACT runs ONLY exp -
     it is the binding engine (~133us); all evacuations are on DVE
  5. PV stationary-E into a single 3-bank tile: 16 chunks of
     [128 n, 65] (g = 4*ci + nsub) packed 7/7/2 per bank; col 64 of
     each chunk accumulates the softmax denominator
  6. combine on DVE: strided reciprocals + scalar_tensor_tensor
     (o1*r1 + o2*(-lam*r2)) -> oc[n, nsub, par, ch]
  7. oc -> oT[ch, n] via DMA transpose (XBAR) - no PE/PSUM cost
  8. proj = oT.T @ Wp: n-half 0 interleaved into the last sweep,
     n-half 1 pipelined per-nsub after the final combine
qkv/V/proj units share a 1-bank PSUM pool (disjoint in time).
"""

import sys

sys.path.insert(0, "/opt/trn_rl_repo")

import numpy as np
import ml_dtypes

B, N, C, H, HD = 4, 1024, 1024, 16, 64
LAMBDA_INIT = 0.8
BF16 = ml_dtypes.bfloat16

_PROG_CACHE = {}

LAG = 3
# combo ci=2g+i: 0=(even,h1) 1=(odd,h1) 2=(even,h2) 3=(odd,h2)
# score row group rg for ci (partition range of the half in qkvT)
RG = [0, 2, 1, 3]
# PV chunk g=4*ci+nsub -> (bank, 65-col slot): 7/7/2 packing
GB = [(g // 7, g % 7) for g in range(16)]
G_START = (0, 7, 14)  # first chunk written in each bank (zeroes it)
G_STOP = (6, 13, 15)  # last chunk written in each bank


def _build_program(debug=False):
    key = ("nc", debug)
    if key in _PROG_CACHE:
        return _PROG_CACHE[key]

    import concourse.mybir as mybir
    import concourse.tile as tile
    from concourse import bacc

    f32 = mybir.dt.float32
    b16 = mybir.dt.bfloat16
    Exp = mybir.ActivationFunctionType.Exp
    MUL = mybir.AluOpType.mult
    ADD = mybir.AluOpType.add

    nc = bacc.Bacc(None)

    # host layouts are partition-major so each DMA is one large transfer
    x_d = nc.dram_tensor("xT", [128, 8, N], b16, kind="ExternalInput")
    # wqkv columns reordered: block j'=2*hp+t (t=0 q, t=1 k), v at 1024:1536
    wqkv_d = nc.dram_tensor("wqkv", [128, 8, 1536], b16, kind="ExternalInput")
    wp_d = nc.dram_tensor("wp", [128, 4, C], b16, kind="ExternalInput")
    neglam_d = nc.dram_tensor("neglam", [128, 1], f32, kind="ExternalInput")
    ident_d = nc.dram_tensor("ident", [128, 128], b16, kind="ExternalInput")
    out_d = nc.dram_tensor("out", [8, 128, C], b16, kind="ExternalOutput")
    if debug:
        dbg_qkvT = nc.dram_tensor("dbg_qkvT", [128, 8, N], b16, kind="ExternalOutput")
        dbg_vsb = nc.dram_tensor("dbg_vsb", [128, 8, 8, 65], b16, kind="ExternalOutput")
        dbg_oc = nc.dram_tensor("dbg_oc", [128, 8, 4, 2, 64], b16, kind="ExternalOutput")
        dbg_oT = nc.dram_tensor("dbg_oT", [128, 4, N], b16, kind="ExternalOutput")

    with tile.TileContext(nc) as tc:
        with (
            tc.tile_pool(name="io", bufs=1) as iopool,
            tc.tile_pool(name="work", bufs=4) as wpool,
            tc.tile_pool(name="esb", bufs=12) as epool,
            tc.tile_pool(name="ocp", bufs=2) as ocpool,
            tc.tile_pool(name="pS", bufs=2, space="PSUM") as pS,
            tc.tile_pool(name="pO", bufs=1, space="PSUM") as pO,
            tc.tile_pool(name="pP", bufs=1, space="PSUM") as pP,
        ):
            xT = iopool.tile([128, 8, N], b16)
            wqkv = iopool.tile([128, 8, 1536], b16)
            wp = iopool.tile([128, 4, C], b16)
            neglam = iopool.tile([128, 1], f32)
            # qkvT chunk j=2*hp+t: partitions 0-63 even head d0..63,
            # 64-127 odd head d0..63
            qkvT = iopool.tile([128, 8, N], b16)
            # V in [keys, channels] layout; col 64 of each head = ones
            vsb = iopool.tile([128, 8, 8, 65], b16)
            # transposed attention out for proj: [ch-part, hp, n]
            oT = iopool.tile([128, 4, N], b16)
            ident = iopool.tile([128, 128], b16)
            if debug:
                dbg_oc_t = iopool.tile([128, 8, 4, 2, 64], b16)

            warm = iopool.tile([128, 256], b16)
            nc.gpsimd.memset(warm[:], 0.5)
            nc.gpsimd.memset(vsb[:, :, :, 64:65], 1.0)

            # ---------------- DMA in (strict need order) ----------------
            # The shared DMA slot grants waiting transfers in ARBITRARY
            # order, so a late-need transfer that queues early can starve a
            # critical one.  Put everything on the sync queue in exact need
            # order - its ~1.3us per-issue pacing self-throttles - except
            # w[j0,j1] which rides the otherwise-empty scalar queue.
            nc.scalar.dma_start(wqkv[:, :, 0:256], wqkv_d[:, :, 0:256])
            for cc2 in range(4):
                nc.sync.dma_start(
                    xT[:, 2 * cc2 : 2 * cc2 + 2, 0:512],
                    x_d[:, 2 * cc2 : 2 * cc2 + 2, 0:512],
                )
            # v weights before xh1: V-u0 consumes them first (it only needs
            # xh0), while xh1's first consumer is the k2 unit a bit later
            nc.sync.dma_start(wqkv[:, :, 1024:1280], wqkv_d[:, :, 1024:1280])
            nc.sync.dma_start(wqkv[:, :, 1280:1536], wqkv_d[:, :, 1280:1536])
            for h4 in range(2):
                nc.sync.dma_start(
                    xT[:, 4 * h4 : 4 * h4 + 4, 512:1024],
                    x_d[:, 4 * h4 : 4 * h4 + 4, 512:1024],
                )
            nc.gpsimd.dma_start(neglam[:], neglam_d[:])
            for hp in range(1, 4):
                c0 = hp * 256
                nc.sync.dma_start(wqkv[:, :, c0 : c0 + 256], wqkv_d[:, :, c0 : c0 + 256])
            nc.sync.dma_start(wp[:], wp_d[:])
            nc.sync.dma_start(ident[:], ident_d[:])

            # PE warm-up during the input DMAs: the pstate model needs ~3us
            # of continuous execution to reach full clock, so burn it on
            # dummy matmuls into a single scratch PSUM tile (one slot alloc
            # so the pP ring is not serialized)
            wps = pP.tile([128, 512], f32, tag="p", name="warm")
            for w in range(13):
                nc.tensor.matmul(
                    wps[0:1, 0:256], warm[:, 0:1], warm[:], start=True, stop=True
                )

            # ---------------- filler units ------------------------------
            def emit_qkv_unit(hp, t, nh, pool=None):
                # one [128,512] q/k projection chunk -> qkvT[:, 2hp+t, nh]
                j = 2 * hp + t
                pool = pool or pP
                ps = pool.tile([128, 512], f32, tag=pool.name[1].lower(),
                               name=f"qkv{j}_{nh}")
                for cc in range(8):
                    nc.tensor.matmul(
                        ps[:],
                        wqkv[:, cc, j * 128 : (j + 1) * 128],
                        xT[:, cc, nh * 512 : (nh + 1) * 512],
                        start=(cc == 0),
                        stop=(cc == 7),
                    )
                nc.vector.tensor_copy(qkvT[:, j, nh * 512 : (nh + 1) * 512], ps[:])

            vparts = {}

            def emit_v_half(mc, part):
                # v -> [keys, channels] layout (operands swapped); DVE evac;
                # emitted in two half-contraction quanta to smooth PE load
                if part == 0:
                    vparts[mc] = pP.tile([128, 512], f32, tag="p", name=f"v{mc}")
                ps = vparts[mc]
                for cc in range(4 * part, 4 * part + 4):
                    nc.tensor.matmul(
                        ps[:],
                        xT[:, cc, mc * 128 : (mc + 1) * 128],
                        wqkv[:, cc, 1024:1536],
                        start=(cc == 0),
                        stop=(cc == 7),
                    )
                if part == 1:
                    nc.vector.tensor_copy(
                        vsb[:, mc, :, 0:64], ps.rearrange("p (g d) -> p g d", g=8)
                    )

            def emit_v_unit(mc):
                emit_v_half(mc, 0)
                emit_v_half(mc, 1)

            def emit_proj_unit(ncc, jh, pool, on_act=False):
                # out[ncc n-chunk, jh 512 out-ch] = oT.T @ wp, K=512 (4 ci)
                # tail units evacuate on ACT (idle once the exps are done)
                ps = pool.tile([128, 512], f32, tag=pool.name[1].lower(),
                               name=f"proj{ncc}_{jh}")
                for ci in range(4):
                    nc.tensor.matmul(
                        ps[:],
                        oT[:, ci, ncc * 128 : (ncc + 1) * 128],
                        wp[:, ci, jh * 512 : (jh + 1) * 512],
                        start=(ci == 0),
                        stop=(ci == 3),
                    )
                osb = wpool.tile([128, 512], b16, tag="osb", bufs=6)
                if on_act:
                    # tail: ACT evac + scalar-queue DMA keeps the sync queue
                    # free for the final transposes' waits
                    nc.scalar.copy(osb[:], ps[:])
                    nc.scalar.dma_start(out_d[ncc, :, jh * 512 : (jh + 1) * 512], osb[:])
                else:
                    nc.vector.tensor_copy(osb[:], ps[:])
                    nc.sync.dma_start(out_d[ncc, :, jh * 512 : (jh + 1) * 512], osb[:])

            projA = {}

            def emit_proj_partial(ncc, jh):
                # n-half-1 proj split: accumulate head-pairs 0-2 into PSUM
                # during sweep 6-7 slack, park the partial in SBUF f32
                ps = pP.tile([128, 512], f32, tag="p", name=f"pp{ncc}_{jh}")
                for ci in range(3):
                    nc.tensor.matmul(
                        ps[:],
                        oT[:, ci, ncc * 128 : (ncc + 1) * 128],
                        wp[:, ci, jh * 512 : (jh + 1) * 512],
                        start=(ci == 0),
                        stop=(ci == 2),
                    )
                pa = wpool.tile([128, 512], f32, tag=f"pa{ncc}_{jh}", bufs=1)
                nc.vector.tensor_copy(pa[:], ps[:])
                projA[(ncc, jh)] = pa

            def emit_proj_final(ncc, jh, pool):
                # tail: one head-pair-3 matmul + DVE add of the parked partial
                ps = pool.tile([128, 512], f32, tag=pool.name[1].lower(),
                               name=f"pf{ncc}_{jh}")
                nc.tensor.matmul(
                    ps[:],
                    oT[:, 3, ncc * 128 : (ncc + 1) * 128],
                    wp[:, 3, jh * 512 : (jh + 1) * 512],
                    start=True,
                    stop=True,
                )
                osb = wpool.tile([128, 512], b16, tag="osb", bufs=6)
                nc.vector.tensor_tensor(osb[:], ps[:], projA[(ncc, jh)][:], ADD)
                nc.sync.dma_start(out_d[ncc, :, jh * 512 : (jh + 1) * 512], osb[:])

            # ---------------- attention pieces --------------------------
            def emit_scores_exp(hp, nh, mc):
                # 4 score matmuls + 2 exps; returns e tiles [g0, g1]
                cur = []
                for g in range(2):
                    s_ps = pS.tile([128, 2, 512], f32, tag="s")
                    for i in range(2):
                        rg = RG[2 * g + i]
                        nc.tensor.matmul(
                            s_ps[:, i, :],
                            qkvT[
                                32 * rg : 32 * rg + 32,
                                2 * hp + 1,
                                mc * 128 : (mc + 1) * 128,
                            ],
                            qkvT[
                                32 * rg : 32 * rg + 32,
                                2 * hp,
                                nh * 512 : (nh + 1) * 512,
                            ],
                            start=True,
                            stop=True,
                            tile_position=(32 * rg, 0),
                        )
                    e_sb = epool.tile([128, 2, 512], b16, tag="e")
                    nc.scalar.activation(e_sb[:], s_ps[:], Exp, scale=0.125)
                    cur.append(e_sb)
                return cur

            def emit_pv(hp, etiles, o, mc):
                # stationary-E PV into the 3-bank packed tile o [128,3,512]
                # chunk g=4*ci+nsub at (bank g//7, col (g%7)*65); start=True
                # zeroes the whole bank so only its first chunk may set it
                for ci in range(4):
                    par = ci % 2
                    for nsub in range(4):
                        g = 4 * ci + nsub
                        b, sl = GB[g]
                        nc.tensor.matmul(
                            o[:, b, sl * 65 : sl * 65 + 65],
                            etiles[ci // 2][:, par, nsub * 128 : (nsub + 1) * 128],
                            vsb[:, mc, 2 * hp + par, :],
                            start=(mc == 0 and g in G_START),
                            stop=(mc == 7 and g in G_STOP),
                            skip_group_check=True,
                        )

            def emit_oraw(o):
                # bulk-evacuate the packed accumulator to SBUF (3 copies,
                # one per bank) so the PSUM tile is released fast - the
                # normalization then runs off the critical path from SBUF
                oraw = wpool.tile([128, 16, 65], f32, tag="oraw")
                orv = oraw.rearrange("p g c -> p (g c)")
                for b, cnt in ((0, 7), (1, 7), (2, 2)):
                    nc.vector.tensor_copy(
                        orv[:, b * 455 : b * 455 + cnt * 65], o[:, b, 0 : cnt * 65]
                    )
                return oraw

            def emit_recips(oraw):
                # per-partition reciprocals of the stride-65 denominators
                r = wpool.tile([128, 16, 1], f32, tag="r")
                nc.vector.reciprocal(r[:], oraw[:, :, 64:65])
                # -lam fold on the h2 chunks (g 8..15)
                nc.vector.tensor_scalar_mul(r[:, 8:16], r[:, 8:16], neglam[:])
                return r

            def emit_combine_chunk(hp, oraw, r, oc, nsub):
                # one nsub (128 n cols): 2 parities, fused on DVE, all SBUF
                for par in range(2):
                    g1 = 4 * par + nsub
                    g2 = 8 + 4 * par + nsub
                    t = wpool.tile([128, 64], f32, tag=f"t{par}")
                    nc.vector.tensor_scalar_mul(
                        t[:], oraw[:, g2, 0:64], r[:, g2]
                    )
                    nc.vector.scalar_tensor_tensor(
                        oc[:, nsub, par, :],
                        oraw[:, g1, 0:64],
                        r[:, g1],
                        t[:],
                        MUL,
                        ADD,
                    )

            def emit_combine(hp, nh, o):
                oraw = emit_oraw(o)
                r = emit_recips(oraw)
                oc = ocpool.tile([128, 4, 2, 64], b16, tag="oc")
                for nsub in range(4):
                    emit_combine_chunk(hp, oraw, r, oc, nsub)
                if debug:
                    nc.vector.tensor_copy(dbg_oc_t[:, 2 * hp + nh], oc[:])
                return oc

            def emit_transpose(hp, nh, oc, nsub):
                # oc[128 n, par, 64ch] -> oT[128 ch, n] via DMA xbar
                n0 = nh * 512 + nsub * 128
                nc.sync.dma_start_transpose(oT[:, hp, n0 : n0 + 128], oc[:, nsub])

            def emit_transpose_pe(hp, nh, oc, nsub):
                # final-sweep transpose on PE (PSUM is free, and the ~2.4us
                # DMA-transpose latency would sit on the critical tail)
                n0 = nh * 512 + nsub * 128
                trb = pS.tile([128, 128], b16, tag="s", name="trb")
                nc.tensor.matmul(trb[:], oc[:, nsub], ident[:], is_transpose=True)
                nc.scalar.copy(oT[:, hp, n0 : n0 + 128], trb[:])

            # ---------------- filler schedule ---------------------------
            filler = {}

            def add(slot, fn, *args):
                filler.setdefault(slot, []).append((fn, args))

            # sweep 0 (hp0,nh0): V units + rest of hp0 qkv.  V-u(k) must be
            # emitted by the fillers of slot k+LAG (its PV consumer); the
            # second-half hp1 units are deferred to sweep 2's slack.
            add((0, 0), emit_v_unit, 0)
            add((0, 1), emit_v_unit, 1)
            add((0, 2), emit_qkv_unit, 0, 1, 1)  # k hp0 keys 512-1023 (mc4+)
            add((0, 3), emit_v_unit, 2)
            add((0, 4), emit_v_unit, 3)
            add((0, 5), emit_qkv_unit, 0, 0, 1)  # q hp0 n 512+ (sweep 1)
            add((0, 6), emit_v_unit, 4)
            add((0, 7), emit_v_unit, 5)
            add((1, 0), emit_v_unit, 6)
            add((1, 0), emit_v_unit, 7)
            # sweep 1: first-half hp1 qkv (needed by sweep 2 start)
            add((1, 2), emit_qkv_unit, 1, 1, 0)
            add((1, 5), emit_qkv_unit, 1, 0, 0)
            # sweep 2: rest of hp1 (k-u2 by slot 4, q-u2 by sweep 3)
            add((2, 0), emit_qkv_unit, 1, 1, 1)
            add((2, 2), emit_qkv_unit, 1, 0, 1)
            # sweeps 2-3: hp2 (needed by sweep 4)
            add((2, 5), emit_qkv_unit, 2, 1, 0)
            add((3, 0), emit_qkv_unit, 2, 1, 1)
            add((3, 3), emit_qkv_unit, 2, 0, 0)
            add((3, 6), emit_qkv_unit, 2, 0, 1)
            # sweeps 4-5: hp3 (needed by sweep 6)
            add((4, 2), emit_qkv_unit, 3, 1, 0)
            add((4, 6), emit_qkv_unit, 3, 1, 1)
            add((5, 2), emit_qkv_unit, 3, 0, 0)
            add((5, 6), emit_qkv_unit, 3, 0, 1)
            # sweep 6: three n-half-1 proj partials (hp0-2) at 2-slot
            # spacing so each pP evac completes before the next alloc
            add((6, 3), emit_proj_partial, 4, 0)
            add((6, 5), emit_proj_partial, 4, 1)
            add((6, 7), emit_proj_partial, 5, 0)
            # sweep 7: proj units for n-half 0 ride the per-slot PE slack
            # (evacs on DVE so they don't steal ACT from the exps).  They
            # must come at mc>=2: sweep 6's transposes land in the mc==1
            # combine block, and a unit emitted before them would read
            # oT[:, 3, :] with no registered writer.
            add((7, 2), emit_proj_unit, 0, 0, pP)
            add((7, 3), emit_proj_unit, 0, 1, pP)
            add((7, 4), emit_proj_unit, 1, 0, pP)
            add((7, 5), emit_proj_unit, 1, 1, pP)
            add((7, 6), emit_proj_unit, 2, 0, pP)
            add((7, 7), emit_proj_unit, 2, 1, pP)

            # qkv for the first sweep must precede it.  k and q interleave
            # per 2-cc chunk (k on pP, q on a free pS slot) so both chase
            # the xh0 chunk arrivals; k keys 0-127 evacuate first since the
            # first score matmul only needs that slice.
            kps = pP.tile([128, 512], f32, tag="p", name="k0pre")
            qps = pS.tile([128, 512], f32, tag="s", name="q0pre")
            for cc in range(8):
                nc.tensor.matmul(
                    kps[:], wqkv[:, cc, 128:256], xT[:, cc, 0:512],
                    start=(cc == 0), stop=(cc == 7),
                )
                nc.tensor.matmul(
                    qps[:], wqkv[:, cc, 0:128], xT[:, cc, 0:512],
                    start=(cc == 0), stop=(cc == 7),
                )
            nc.vector.tensor_copy(qkvT[:, 1, 0:128], kps[:, 0:128])
            nc.vector.tensor_copy(qkvT[:, 0, 0:512], qps[:])
            nc.vector.tensor_copy(qkvT[:, 1, 128:512], kps[:, 128:512])

            # ---------------- main pipeline -----------------------------
            # software-pipelined across sweep boundaries: the tail PVs of
            # sweep s are emitted after sweep s+1's first scores/exp so the
            # next exp is never behind them on PE; combine lands at mc==1
            pending = None  # (hp, nh, o_tile)
            tailpv = None  # (hp, edeque, o_tile)
            for s in range(8):
                hp, nh = s // 2, s % 2
                o = None
                edeque = []
                for mc in range(8):
                    edeque.append(emit_scores_exp(hp, nh, mc))
                    for fn, args in filler.get((s, mc), ()):
                        fn(*args)
                    if mc == 0 and tailpv is not None:
                        for mcp in range(8 - LAG, 8):
                            emit_pv(tailpv[0], tailpv[1][mcp], tailpv[2], mcp)
                        tailpv = None
                    if mc == 1 and pending is not None:
                        oc = emit_combine(*pending)
                        for nsub in range(4):
                            emit_transpose(pending[0], pending[1], oc, nsub)
                        pending = None
                    if mc >= LAG:
                        if o is None:
                            o = pO.tile([128, 3, 512], f32, tag="o", name=f"o{s}")
                        emit_pv(hp, edeque[mc - LAG], o, mc - LAG)
                pending = (hp, nh, o)
                tailpv = (hp, edeque, o)

            # final sweep: tail PVs, then chunked combine -> transpose ->
            # proj pipeline; two held-back n-half-0 proj units keep PE warm
            hp, nh, o = pending
            for mcp in range(8 - LAG, 8):
                emit_pv(hp, tailpv[1][mcp], o, mcp)
            oraw = emit_oraw(o)
            r = emit_recips(oraw)
            # held-back n-half-0 proj units fill the combine window on PE;
            # evacs stay on DVE so ACT only carries the tr copies and the
            # two engines pipeline the tail instead of serializing on ACT
            emit_proj_unit(3, 0, pP)
            emit_proj_unit(3, 1, pS)
            oc = ocpool.tile([128, 4, 2, 64], b16, tag="oc")
            for nsub in range(4):
                emit_combine_chunk(hp, oraw, r, oc, nsub)
                emit_transpose_pe(hp, nh, oc, nsub)
                if (4 + nsub, 0) in projA:
                    emit_proj_final(4 + nsub, 0, pP if nsub % 2 else pS)
                else:
                    emit_proj_unit(4 + nsub, 0, pP if nsub % 2 else pS)
                # last group: second evac on the now-idle ACT engine so the
                # two final evac+DMA chains run on parallel engines
                if (4 + nsub, 1) in projA:
                    emit_proj_final(4 + nsub, 1, pS if nsub % 2 else pP)
                else:
                    emit_proj_unit(4 + nsub, 1, pS if nsub % 2 else pP,
                                   on_act=(nsub == 3))
            if debug:
                nc.vector.tensor_copy(dbg_oc_t[:, 2 * hp + nh], oc[:])

            if debug:
                nc.sync.dma_start(dbg_qkvT[:], qkvT[:])
                nc.sync.dma_start(dbg_vsb[:], vsb[:])
                nc.sync.dma_start(dbg_oc[:], dbg_oc_t[:])
                nc.sync.dma_start(dbg_oT[:], oT[:])

    nc.compile()
    _PROG_CACHE[key] = nc
    return nc


def _prep_core_inputs(x, W_qkv, W_proj, neg_lam):
    """Host-side shard prep. Returns in_maps for the 8 cores."""
    W4 = np.asarray(W_qkv, np.float32).reshape(3, H, HD, C)
    in_maps = []
    for core in range(8):
        b, hg = divmod(core, 2)
        xT = (
            np.ascontiguousarray(np.asarray(x[b], np.float32).T)
            .reshape(8, 128, N)
            .transpose(1, 0, 2)
            .astype(BF16)
        )
        wsl = W4[:, hg * 8 : (hg + 1) * 8]  # [3, 8 heads, 64, 1024]
        # columns [t(3), head(8), d(64)]; reorder q/k to j'=2*hp+t blocks
        Wcols = np.ascontiguousarray(wsl.transpose(3, 0, 1, 2).reshape(C, 1536))
        Wnew = np.empty_like(Wcols)
        for hp in range(4):
            for t in range(2):
                src = t * 512 + hp * 128
                dst = (2 * hp + t) * 128
                Wnew[:, dst : dst + 128] = Wcols[:, src : src + 128]
        Wnew[:, 1024:1536] = Wcols[:, 1024:1536]
        wqkv = Wnew.reshape(8, 128, 1536).transpose(1, 0, 2).astype(BF16)
        wp = (
            np.ascontiguousarray(
                np.asarray(W_proj, np.float32)[:, hg * 512 : (hg + 1) * 512].T
            )
            .reshape(4, 128, C)
            .transpose(1, 0, 2)
            .astype(BF16)
        )
        in_maps.append(
            {
                "xT": np.ascontiguousarray(xT),
                "wqkv": np.ascontiguousarray(wqkv),
                "wp": np.ascontiguousarray(wp),
                "neglam": np.full((128, 1), neg_lam, np.float32),
                "ident": np.eye(128, dtype=np.float32).astype(BF16),
            }
        )
    return in_maps


def kernel(x, W_qkv, W_proj, b_proj, lambda_q1, lambda_k1, lambda_q2, lambda_k2):
    from concourse.bass_utils import run_bass_kernel_spmd

    lq1 = np.asarray(lambda_q1, np.float64)
    lk1 = np.asarray(lambda_k1, np.float64)
    lq2 = np.asarray(lambda_q2, np.float64)
    lk2 = np.asarray(lambda_k2, np.float64)
    lam = float(np.mean(np.exp(lq1 * lk1) - np.exp(lq2 * lk2) + LAMBDA_INIT))

    nc = _build_program()
    in_maps = _prep_core_inputs(x, W_qkv, W_proj, -lam)
    res = run_bass_kernel_spmd(nc, in_maps, core_ids=list(range(8)))
    _PROG_CACHE["last_result"] = res

    bp = np.asarray(b_proj, np.float32)
    out = np.empty((B, N, C), np.float32)
    for b in range(B):
        p0 = res.results[2 * b]["out"].astype(np.float32).reshape(N, C)
        p1 = res.results[2 * b + 1]["out"].astype(np.float32).reshape(N, C)
        out[b] = p0 + p1 + bp[None, :]
    return out


# revision 65
# speedup vs baseline: 1.0018x; 1.0004x over previous
"""DifferentialAttention Trainium2 kernel (8 NeuronCores, SPMD).

Sharding: data-parallel over batch B=4, tensor-parallel over heads
(2 cores per batch element, 8 heads each).  Each core computes the
partial projection output for its 8 heads; the host sums the two
bf16 partials per batch element in f32 and adds b_proj.

Per-core pipeline (bf16 matmuls, fp32 PSUM), 8 sweeps of
(head-pair hp, 512-col n-half nh):
  1. QKV^T = W_slice^T.T @ x^T            -> [channels, n] layout
  2. V via PE with swapped operands        -> [keys, ch|1] layout
  3. scores S^T[keys, n] per (head, half) with 4-way row groups;
     PSUM writes bank-aligned (tile_position requires it)
  4. exp on ACT (scale=1/8 folded), bf16 out; ACT runs ONLY exp -
     it is the binding engine (~133us); all evacuations are on DVE
  5. PV stationary-E into a single 3-bank tile: 16 chunks of
     [128 n, 65] (g = 4*ci + nsub) packed 7/7/2 per bank; col 64 of
     each chunk accumulates the softmax denominator
  6. combine on DVE: strided reciprocals + scalar_tensor_tensor
     (o1*r1 + o2*(-lam*r2)) -> oc[n, nsub, par, ch]
  7. oc -> oT[ch, n] via DMA transpose (XBAR) - no PE/PSUM cost
  8. proj = oT.T @ Wp: n-half 0 interleaved into the last sweep,
     n-half 1 pipelined per-nsub after the final combine
qkv/V/proj units share a 1-bank PSUM pool (disjoint in time).
"""

import sys

sys.path.insert(0, "/opt/trn_rl_repo")

import numpy as np
import ml_dtypes

B, N, C, H, HD = 4, 1024, 1024, 16, 64
LAMBDA_INIT = 0.8
BF16 = ml_dtypes.bfloat16

_PROG_CACHE = {}

LAG = 3
# combo ci=2g+i: 0=(even,h1) 1=(odd,h1) 2=(even,h2) 3=(odd,h2)
# score row group rg for ci (partition range of the half in qkvT)
RG = [0, 2, 1, 3]
# PV chunk g=4*ci+nsub -> (bank, 65-col slot): 7/7/2 packing
GB = [(g // 7, g % 7) for g in range(16)]
G_START = (0, 7, 14)  # first chunk written in each bank (zeroes it)
G_STOP = (6, 13, 15)  # last chunk written in each bank


def _build_program(debug=False):
    key = ("nc", debug)
    if key in _PROG_CACHE:
        return _PROG_CACHE[key]

    import concourse.mybir as mybir
    import concourse.tile as tile
    from concourse import bacc

    f32 = mybir.dt.float32
    b16 = mybir.dt.bfloat16
    Exp = mybir.ActivationFunctionType.Exp
    MUL = mybir.AluOpType.mult
    ADD = mybir.AluOpType.add

    nc = bacc.Bacc(None)

    # host layouts are partition-major so each DMA is one large transfer
    x_d = nc.dram_tensor("xT", [128, 8, N], b16, kind="ExternalInput")
    # wqkv columns reordered: block j'=2*hp+t (t=0 q, t=1 k), v at 1024:1536
    wqkv_d = nc.dram_tensor("wqkv", [128, 8, 1536], b16, kind="ExternalInput")
    wp_d = nc.dram_tensor("wp", [128, 4, C], b16, kind="ExternalInput")
    neglam_d = nc.dram_tensor("neglam", [128, 1], f32, kind="ExternalInput")
    ident_d = nc.dram_tensor("ident", [128, 128], b16, kind="ExternalInput")
    out_d = nc.dram_tensor("out", [8, 128, C], b16, kind="ExternalOutput")
    if debug:
        dbg_qkvT = nc.dram_tensor("dbg_qkvT", [128, 8, N], b16, kind="ExternalOutput")
        dbg_vsb = nc.dram_tensor("dbg_vsb", [128, 8, 8, 65], b16, kind="ExternalOutput")
        dbg_oc = nc.dram_tensor("dbg_oc", [128, 8, 4, 2, 64], b16, kind="ExternalOutput")
        dbg_oT = nc.dram_tensor("dbg_oT", [128, 4, N], b16, kind="ExternalOutput")

    with tile.TileContext(nc) as tc:
        with (
            tc.tile_pool(name="io", bufs=1) as iopool,
            tc.tile_pool(name="work", bufs=4) as wpool,
            tc.tile_pool(name="esb", bufs=12) as epool,
            tc.tile_pool(name="ocp", bufs=2) as ocpool,
            tc.tile_pool(name="pS", bufs=2, space="PSUM") as pS,
            tc.tile_pool(name="pO", bufs=1, space="PSUM") as pO,
            tc.tile_pool(name="pP", bufs=1, space="PSUM") as pP,
        ):
            xT = iopool.tile([128, 8, N], b16)
            wqkv = iopool.tile([128, 8, 1536], b16)
            wp = iopool.tile([128, 4, C], b16)
            neglam = iopool.tile([128, 1], f32)
            # qkvT chunk j=2*hp+t: partitions 0-63 even head d0..63,
            # 64-127 odd head d0..63
            qkvT = iopool.tile([128, 8, N], b16)
            # V in [keys, channels] layout; col 64 of each head = ones
            vsb = iopool.tile([128, 8, 8, 65], b16)
            # transposed attention out for proj: [ch-part, hp, n]
            oT = iopool.tile([128, 4, N], b16)
            ident = iopool.tile([128, 128], b16)
            if debug:
                dbg_oc_t = iopool.tile([128, 8, 4, 2, 64], b16)

            warm = iopool.tile([128, 256], b16)
            nc.gpsimd.memset(warm[:], 0.5)
            nc.gpsimd.memset(vsb[:, :, :, 64:65], 1.0)

            # ---------------- DMA in (strict need order) ----------------
            # The shared DMA slot grants waiting transfers in ARBITRARY
            # order, so a late-need transfer that queues early can starve a
            # critical one.  Put everything on the sync queue in exact need
            # order - its ~1.3us per-issue pacing self-throttles - except
            # w[j0,j1] which rides the otherwise-empty scalar queue.
            nc.scalar.dma_start(wqkv[:, :, 0:256], wqkv_d[:, :, 0:256])
            for cc2 in range(4):
                nc.sync.dma_start(
                    xT[:, 2 * cc2 : 2 * cc2 + 2, 0:512],
                    x_d[:, 2 * cc2 : 2 * cc2 + 2, 0:512],
                )
            # v weights before xh1: V-u0 consumes them first (it only needs
            # xh0), while xh1's first consumer is the k2 unit a bit later
            nc.sync.dma_start(wqkv[:, :, 1024:1280], wqkv_d[:, :, 1024:1280])
            nc.sync.dma_start(wqkv[:, :, 1280:1536], wqkv_d[:, :, 1280:1536])
            for h4 in range(2):
                nc.sync.dma_start(
                    xT[:, 4 * h4 : 4 * h4 + 4, 512:1024],
                    x_d[:, 4 * h4 : 4 * h4 + 4, 512:1024],
                )
            nc.gpsimd.dma_start(neglam[:], neglam_d[:])
            for hp in range(1, 4):
                c0 = hp * 256
                nc.sync.dma_start(wqkv[:, :, c0 : c0 + 256], wqkv_d[:, :, c0 : c0 + 256])
            nc.sync.dma_start(wp[:], wp_d[:])
            nc.sync.dma_start(ident[:], ident_d[:])

            # PE warm-up during the input DMAs: the pstate model needs ~3us
            # of continuous execution to reach full clock, so burn it on
            # dummy matmuls into a single scratch PSUM tile (one slot alloc
            # so the pP ring is not serialized)
            wps = pP.tile([128, 512], f32, tag="p", name="warm")
            for w in range(13):
                nc.tensor.matmul(
                    wps[0:1, 0:256], warm[:, 0:1], warm[:], start=True, stop=True
                )

            # ---------------- filler units ------------------------------
            def emit_qkv_unit(hp, t, nh, pool=None):
                # one [128,512] q/k projection chunk -> qkvT[:, 2hp+t, nh]
                j = 2 * hp + t
                pool = pool or pP
                ps = pool.tile([128, 512], f32, tag=pool.name[1].lower(),
                               name=f"qkv{j}_{nh}")
                for cc in range(8):
                    nc.tensor.matmul(
                        ps[:],
                        wqkv[:, cc, j * 128 : (j + 1) * 128],
                        xT[:, cc, nh * 512 : (nh + 1) * 512],
                        start=(cc == 0),
                        stop=(cc == 7),
                    )
                nc.vector.tensor_copy(qkvT[:, j, nh * 512 : (nh + 1) * 512], ps[:])

            vparts = {}

            def emit_v_half(mc, part):
                # v -> [keys, channels] layout (operands swapped); DVE evac;
                # emitted in two half-contraction quanta to smooth PE load
                if part == 0:
                    vparts[mc] = pP.tile([128, 512], f32, tag="p", name=f"v{mc}")
                ps = vparts[mc]
                for cc in range(4 * part, 4 * part + 4):
                    nc.tensor.matmul(
                        ps[:],
                        xT[:, cc, mc * 128 : (mc + 1) * 128],
                        wqkv[:, cc, 1024:1536],
                        start=(cc == 0),
                        stop=(cc == 7),
                    )
                if part == 1:
                    nc.vector.tensor_copy(
                        vsb[:, mc, :, 0:64], ps.rearrange("p (g d) -> p g d", g=8)
                    )

            def emit_v_unit(mc):
                emit_v_half(mc, 0)
                emit_v_half(mc, 1)

            def emit_proj_unit(ncc, jh, pool, on_act=False):
                # out[ncc n-chunk, jh 512 out-ch] = oT.T @ wp, K=512 (4 ci)
                # tail units evacuate on ACT (idle once the exps are done)
                ps = pool.tile([128, 512], f32, tag=pool.name[1].lower(),
                               name=f"proj{ncc}_{jh}")
                for ci in range(4):
                    nc.tensor.matmul(
                        ps[:],
                        oT[:, ci, ncc * 128 : (ncc + 1) * 128],
                        wp[:, ci, jh * 512 : (jh + 1) * 512],
                        start=(ci == 0),
                        stop=(ci == 3),
                    )
                osb = wpool.tile([128, 512], b16, tag="osb", bufs=6)
                if on_act:
                    # tail: ACT evac + scalar-queue DMA keeps the sync queue
                    # free for the final transposes' waits
                    nc.scalar.copy(osb[:], ps[:])
                    nc.scalar.dma_start(out_d[ncc, :, jh * 512 : (jh + 1) * 512], osb[:])
                else:
                    nc.vector.tensor_copy(osb[:], ps[:])
                    nc.sync.dma_start(out_d[ncc, :, jh * 512 : (jh + 1) * 512], osb[:])

            projA = {}

            def emit_proj_partial(ncc, jh):
                # n-half-1 proj split: accumulate head-pairs 0-2 into PSUM
                # during sweep 6-7 slack, park the partial in SBUF f32
                ps = pP.tile([128, 512], f32, tag="p", name=f"pp{ncc}_{jh}")
                for ci in range(3):
                    nc.tensor.matmul(
                        ps[:],
                        oT[:, ci, ncc * 128 : (ncc + 1) * 128],
                        wp[:, ci, jh * 512 : (jh + 1) * 512],
                        start=(ci == 0),
                        stop=(ci == 2),
                    )
                pa = wpool.tile([128, 512], f32, tag=f"pa{ncc}_{jh}", bufs=1)
                nc.vector.tensor_copy(pa[:], ps[:])
                projA[(ncc, jh)] = pa

            def emit_proj_final(ncc, jh, pool):
                # tail: one head-pair-3 matmul + DVE add of the parked partial
                ps = pool.tile([128, 512], f32, tag=pool.name[1].lower(),
                               name=f"pf{ncc}_{jh}")
                nc.tensor.matmul(
                    ps[:],
                    oT[:, 3, ncc * 128 : (ncc + 1) * 128],
                    wp[:, 3, jh * 512 : (jh + 1) * 512],
                    start=True,
                    stop=True,
                )
                osb = wpool.tile([128, 512], b16, tag="osb", bufs=6)
                nc.vector.tensor_tensor(osb[:], ps[:], projA[(ncc, jh)][:], ADD)
                nc.sync.dma_start(out_d[ncc, :, jh * 512 : (jh + 1) * 512], osb[:])

            # ---------------- attention pieces --------------------------
            def emit_scores_exp(hp, nh, mc):
                # 4 score matmuls + 2 exps; returns e tiles [g0, g1]
                cur = []
                for g in range(2):
                    s_ps = pS.tile([128, 2, 512], f32, tag="s")
                    for i in range(2):
                        rg = RG[2 * g + i]
                        nc.tensor.matmul(
                            s_ps[:, i, :],
                            qkvT[
                                32 * rg : 32 * rg + 32,
                                2 * hp + 1,
                                mc * 128 : (mc + 1) * 128,
                            ],
                            qkvT[
                                32 * rg : 32 * rg + 32,
                                2 * hp,
                                nh * 512 : (nh + 1) * 512,
                            ],
                            start=True,
                            stop=True,
                            tile_position=(32 * rg, 0),
                        )
                    e_sb = epool.tile([128, 2, 512], b16, tag="e")
                    nc.scalar.activation(e_sb[:], s_ps[:], Exp, scale=0.125)
                    cur.append(e_sb)
                return cur

            def emit_pv(hp, etiles, o, mc):
                # stationary-E PV into the 3-bank packed tile o [128,3,512]
                # chunk g=4*ci+nsub at (bank g//7, col (g%7)*65); start=True
                # zeroes the whole bank so only its first chunk may set it
                for ci in range(4):
                    par = ci % 2
                    for nsub in range(4):
                        g = 4 * ci + nsub
                        b, sl = GB[g]
                        nc.tensor.matmul(
                            o[:, b, sl * 65 : sl * 65 + 65],
                            etiles[ci // 2][:, par, nsub * 128 : (nsub + 1) * 128],
                            vsb[:, mc, 2 * hp + par, :],
                            start=(mc == 0 and g in G_START),
                            stop=(mc == 7 and g in G_STOP),
                            skip_group_check=True,
                        )

            def emit_oraw(o):
                # bulk-evacuate the packed accumulator to SBUF (3 copies,
                # one per bank) so the PSUM tile is released fast - the
                # normalization then runs off the critical path from SBUF
                oraw = wpool.tile([128, 16, 65], f32, tag="oraw")
                orv = oraw.rearrange("p g c -> p (g c)")
                for b, cnt in ((0, 7), (1, 7), (2, 2)):
                    nc.vector.tensor_copy(
                        orv[:, b * 455 : b * 455 + cnt * 65], o[:, b, 0 : cnt * 65]
                    )
                return oraw

            def emit_recips(oraw):
                # per-partition reciprocals of the stride-65 denominators
                r = wpool.tile([128, 16, 1], f32, tag="r")
                nc.vector.reciprocal(r[:], oraw[:, :, 64:65])
                # -lam fold on the h2 chunks (g 8..15)
                nc.vector.tensor_scalar_mul(r[:, 8:16], r[:, 8:16], neglam[:])
                return r

            def emit_combine_chunk(hp, oraw, r, oc, nsub):
                # one nsub (128 n cols): 2 parities, fused on DVE, all SBUF
                for par in range(2):
                    g1 = 4 * par + nsub
                    g2 = 8 + 4 * par + nsub
                    t = wpool.tile([128, 64], f32, tag=f"t{par}")
                    nc.vector.tensor_scalar_mul(
                        t[:], oraw[:, g2, 0:64], r[:, g2]
                    )
                    nc.vector.scalar_tensor_tensor(
                        oc[:, nsub, par, :],
                        oraw[:, g1, 0:64],
                        r[:, g1],
                        t[:],
                        MUL,
                        ADD,
                    )

            def emit_combine(hp, nh, o):
                oraw = emit_oraw(o)
                r = emit_recips(oraw)
                oc = ocpool.tile([128, 4, 2, 64], b16, tag="oc")
                for nsub in range(4):
                    emit_combine_chunk(hp, oraw, r, oc, nsub)
                if debug:
                    nc.vector.tensor_copy(dbg_oc_t[:, 2 * hp + nh], oc[:])
                return oc

            def emit_transpose(hp, nh, oc, nsub):
                # oc[128 n, par, 64ch] -> oT[128 ch, n] via DMA xbar
                n0 = nh * 512 + nsub * 128
                nc.sync.dma_start_transpose(oT[:, hp, n0 : n0 + 128], oc[:, nsub])

            def emit_transpose_pe(hp, nh, oc, nsub):
                # final-sweep transpose on PE (PSUM is free, and the ~2.4us
                # DMA-transpose latency would sit on the critical tail)
                n0 = nh * 512 + nsub * 128
                trb = pS.tile([128, 128], b16, tag="s", name="trb")
                nc.tensor.matmul(trb[:], oc[:, nsub], ident[:], is_transpose=True)
                nc.scalar.copy(oT[:, hp, n0 : n0 + 128], trb[:])

            # ---------------- filler schedule ---------------------------
            filler = {}

            def add(slot, fn, *args):
                filler.setdefault(slot, []).append((fn, args))

            # sweep 0 (hp0,nh0): V units + rest of hp0 qkv.  V-u(k) must be
            # emitted by the fillers of slot k+LAG (its PV consumer); the
            # second-half hp1 units are deferred to sweep 2's slack.
            add((0, 0), emit_v_unit, 0)
            add((0, 1), emit_v_unit, 1)
            add((0, 2), emit_qkv_unit, 0, 1, 1)  # k hp0 keys 512-1023 (mc4+)
            add((0, 3), emit_v_unit, 2)
            add((0, 4), emit_v_unit, 3)
            add((0, 5), emit_qkv_unit, 0, 0, 1)  # q hp0 n 512+ (sweep 1)
            add((0, 6), emit_v_unit, 4)
            add((0, 7), emit_v_unit, 5)
            add((1, 0), emit_v_unit, 6)
            add((1, 0), emit_v_unit, 7)
            # sweep 1: first-half hp1 qkv (needed by sweep 2 start)
            add((1, 2), emit_qkv_unit, 1, 1, 0)
            add((1, 5), emit_qkv_unit, 1, 0, 0)
            # sweep 2: rest of hp1 (k-u2 by slot 4, q-u2 by sweep 3)
            add((2, 0), emit_qkv_unit, 1, 1, 1)
            add((2, 2), emit_qkv_unit, 1, 0, 1)
            # sweeps 2-3: hp2 (needed by sweep 4)
            add((2, 5), emit_qkv_unit, 2, 1, 0)
            add((3, 0), emit_qkv_unit, 2, 1, 1)
            add((3, 3), emit_qkv_unit, 2, 0, 0)
            add((3, 6), emit_qkv_unit, 2, 0, 1)
            # sweeps 4-5: hp3 (needed by sweep 6)
            add((4, 2), emit_qkv_unit, 3, 1, 0)
            add((4, 6), emit_qkv_unit, 3, 1, 1)
            add((5, 2), emit_qkv_unit, 3, 0, 0)
            add((5, 6), emit_qkv_unit, 3, 0, 1)
            # sweep 6: three n-half-1 proj partials (hp0-2) at 2-slot
            # spacing so each pP evac completes before the next alloc
            add((6, 3), emit_proj_partial, 4, 0)
            add((6, 4), emit_proj_partial, 4, 1)
            add((6, 5), emit_proj_partial, 5, 0)
            add((6, 6), emit_proj_partial, 5, 1)
            add((6, 7), emit_proj_partial, 6, 0)
            # sweep 7: proj units for n-half 0 ride the per-slot PE slack
            # (evacs on DVE so they don't steal ACT from the exps).  They
            # must come at mc>=2: sweep 6's transposes land in the mc==1
            # combine block, and a unit emitted before them would read
            # oT[:, 3, :] with no registered writer.
            add((7, 2), emit_proj_unit, 0, 0, pP)
            add((7, 3), emit_proj_unit, 0, 1, pP)
            add((7, 4), emit_proj_unit, 1, 0, pP)
            add((7, 5), emit_proj_unit, 1, 1, pP)
            add((7, 6), emit_proj_unit, 2, 0, pP)
            add((7, 7), emit_proj_unit, 2, 1, pP)

            # qkv for the first sweep must precede it.  k and q interleave
            # per 2-cc chunk (k on pP, q on a free pS slot) so both chase
            # the xh0 chunk arrivals; k keys 0-127 evacuate first since the
            # first score matmul only needs that slice.
            kps = pP.tile([128, 512], f32, tag="p", name="k0pre")
            qps = pS.tile([128, 512], f32, tag="s", name="q0pre")
            for cc in range(8):
                nc.tensor.matmul(
                    kps[:], wqkv[:, cc, 128:256], xT[:, cc, 0:512],
                    start=(cc == 0), stop=(cc == 7),
                )
                nc.tensor.matmul(
                    qps[:], wqkv[:, cc, 0:128], xT[:, cc, 0:512],
                    start=(cc == 0), stop=(cc == 7),
                )
            nc.vector.tensor_copy(qkvT[:, 1, 0:128], kps[:, 0:128])
            nc.vector.tensor_copy(qkvT[:, 0, 0:512], qps[:])
            nc.vector.tensor_copy(qkvT[:, 1, 128:512], kps[:, 128:512])

            # ---------------- main pipeline -----------------------------
            # software-pipelined across sweep boundaries: the tail PVs of
            # sweep s are emitted after sweep s+1's first scores/exp so the
            # next exp is never behind them on PE; combine lands at mc==1
            pending = None  # (hp, nh, o_tile)
            tailpv = None  # (hp, edeque, o_tile)
            for s in range(8):
                hp, nh = s // 2, s % 2
                o = None
                edeque = []
                for mc in range(8):
                    edeque.append(emit_scores_exp(hp, nh, mc))
                    for fn, args in filler.get((s, mc), ()):
                        fn(*args)
                    if mc == 0 and tailpv is not None:
                        for mcp in range(8 - LAG, 8):
                            emit_pv(tailpv[0], tailpv[1][mcp], tailpv[2], mcp)
                        tailpv = None
                    if mc == 1 and pending is not None:
                        oc = emit_combine(*pending)
                        for nsub in range(4):
                            emit_transpose(pending[0], pending[1], oc, nsub)
                        pending = None
                    if mc >= LAG:
                        if o is None:
                            o = pO.tile([128, 3, 512], f32, tag="o", name=f"o{s}")
                        emit_pv(hp, edeque[mc - LAG], o, mc - LAG)
                pending = (hp, nh, o)
                tailpv = (hp, edeque, o)

            # final sweep: tail PVs, then chunked combine -> transpose ->
            # proj pipeline; two held-back n-half-0 proj units keep PE warm
            hp, nh, o = pending
            for mcp in range(8 - LAG, 8):
                emit_pv(hp, tailpv[1][mcp], o, mcp)
            oraw = emit_oraw(o)
            r = emit_recips(oraw)
            # held-back n-half-0 proj units fill the combine window on PE;
            # evacs stay on DVE so ACT only carries the tr copies and the
            # two engines pipeline the tail instead of serializing on ACT
            emit_proj_unit(3, 0, pP)
            emit_proj_unit(3, 1, pS)
            oc = ocpool.tile([128, 4, 2, 64], b16, tag="oc")
            for nsub in range(4):
                emit_combine_chunk(hp, oraw, r, oc, nsub)
                emit_transpose_pe(hp, nh, oc, nsub)
                if (4 + nsub, 0) in projA:
                    emit_proj_final(4 + nsub, 0, pP if nsub % 2 else pS)
                else:
                    emit_proj_unit(4 + nsub, 0, pP if nsub % 2 else pS)
                # last group: second evac on the now-idle ACT engine so the
                # two final evac+DMA chains run on parallel engines
                if (4 + nsub, 1) in projA:
                    emit_proj_final(4 + nsub, 1, pS if nsub % 2 else pP)
                else:
                    emit_proj_unit(4 + nsub, 1, pS if nsub % 2 else pP,
                                   on_act=(nsub == 3))
            if debug:
                nc.vector.tensor_copy(dbg_oc_t[:, 2 * hp + nh], oc[:])

            if debug:
                nc.sync.dma_start(dbg_qkvT[:], qkvT[:])
                nc.sync.dma_start(dbg_vsb[:], vsb[:])
                nc.sync.dma_start(dbg_oc[:], dbg_oc_t[:])
                nc.sync.dma_start(dbg_oT[:], oT[:])

    nc.compile()
    _PROG_CACHE[key] = nc
    return nc


def _prep_core_inputs(x, W_qkv, W_proj, neg_lam):
    """Host-side shard prep. Returns in_maps for the 8 cores."""
    W4 = np.asarray(W_qkv, np.float32).reshape(3, H, HD, C)
    in_maps = []
    for core in range(8):
        b, hg = divmod(core, 2)
        xT = (
            np.ascontiguousarray(np.asarray(x[b], np.float32).T)
            .reshape(8, 128, N)
            .transpose(1, 0, 2)
            .astype(BF16)
        )
        wsl = W4[:, hg * 8 : (hg + 1) * 8]  # [3, 8 heads, 64, 1024]
        # columns [t(3), head(8), d(64)]; reorder q/k to j'=2*hp+t blocks
        Wcols = np.ascontiguousarray(wsl.transpose(3, 0, 1, 2).reshape(C, 1536))
        Wnew = np.empty_like(Wcols)
        for hp in range(4):
            for t in range(2):
                src = t * 512 + hp * 128
                dst = (2 * hp + t) * 128
                Wnew[:, dst : dst + 128] = Wcols[:, src : src + 128]
        Wnew[:, 1024:1536] = Wcols[:, 1024:1536]
        wqkv = Wnew.reshape(8, 128, 1536).transpose(1, 0, 2).astype(BF16)
        wp = (
            np.ascontiguousarray(
                np.asarray(W_proj, np.float32)[:, hg * 512 : (hg + 1) * 512].T
            )
            .reshape(4, 128, C)
            .transpose(1, 0, 2)
            .astype(BF16)
        )
        in_maps.append(
            {
                "xT": np.ascontiguousarray(xT),
                "wqkv": np.ascontiguousarray(wqkv),
                "wp": np.ascontiguousarray(wp),
                "neglam": np.full((128, 1), neg_lam, np.float32),
                "ident": np.eye(128, dtype=np.float32).astype(BF16),
            }
        )
    return in_maps


def kernel(x, W_qkv, W_proj, b_proj, lambda_q1, lambda_k1, lambda_q2, lambda_k2):
    from concourse.bass_utils import run_bass_kernel_spmd

    lq1 = np.asarray(lambda_q1, np.float64)
    lk1 = np.asarray(lambda_k1, np.float64)
    lq2 = np.asarray(lambda_q2, np.float64)
    lk2 = np.asarray(lambda_k2, np.float64)
    lam = float(np.mean(np.exp(lq1 * lk1) - np.exp(lq2 * lk2) + LAMBDA_INIT))

    nc = _build_program()
    in_maps = _prep_core_inputs(x, W_qkv, W_proj, -lam)
    res = run_bass_kernel_spmd(nc, in_maps, core_ids=list(range(8)))
    _PROG_CACHE["last_result"] = res

    bp = np.asarray(b_proj, np.float32)
    out = np.empty((B, N, C), np.float32)
    for b in range(B):
        p0 = res.results[2 * b]["out"].astype(np.float32).reshape(N, C)
        p1 = res.results[2 * b + 1]["out"].astype(np.float32).reshape(N, C)
        out[b] = p0 + p1 + bp[None, :]
    return out


# revision 66
# speedup vs baseline: 1.0040x; 1.0022x over previous
"""DifferentialAttention Trainium2 kernel (8 NeuronCores, SPMD).

Sharding: data-parallel over batch B=4, tensor-parallel over heads
(2 cores per batch element, 8 heads each).  Each core computes the
partial projection output for its 8 heads; the host sums the two
bf16 partials per batch element in f32 and adds b_proj.

Per-core pipeline (bf16 matmuls, fp32 PSUM), 8 sweeps of
(head-pair hp, 512-col n-half nh):
  1. QKV^T = W_slice^T.T @ x^T            -> [channels, n] layout
  2. V via PE with swapped operands        -> [keys, ch|1] layout
  3. scores S^T[keys, n] per (head, half) with 4-way row groups;
     PSUM writes bank-aligned (tile_position requires it)
  4. exp on ACT (scale=1/8 folded), bf16 out; ACT runs ONLY exp -
     it is the binding engine (~133us); all evacuations are on DVE
  5. PV stationary-E into a single 3-bank tile: 16 chunks of
     [128 n, 65] (g = 4*ci + nsub) packed 7/7/2 per bank; col 64 of
     each chunk accumulates the softmax denominator
  6. combine on DVE: strided reciprocals + scalar_tensor_tensor
     (o1*r1 + o2*(-lam*r2)) -> oc[n, nsub, par, ch]
  7. oc -> oT[ch, n] via DMA transpose (XBAR) - no PE/PSUM cost
  8. proj = oT.T @ Wp: n-half 0 interleaved into the last sweep,
     n-half 1 pipelined per-nsub after the final combine
qkv/V/proj units share a 1-bank PSUM pool (disjoint in time).
"""

import sys

sys.path.insert(0, "/opt/trn_rl_repo")

import numpy as np
import ml_dtypes

B, N, C, H, HD = 4, 1024, 1024, 16, 64
LAMBDA_INIT = 0.8
BF16 = ml_dtypes.bfloat16

_PROG_CACHE = {}

LAG = 3
# combo ci=2g+i: 0=(even,h1) 1=(odd,h1) 2=(even,h2) 3=(odd,h2)
# score row group rg for ci (partition range of the half in qkvT)
RG = [0, 2, 1, 3]
# PV chunk g=4*ci+nsub -> (bank, 65-col slot): 7/7/2 packing
GB = [(g // 7, g % 7) for g in range(16)]
G_START = (0, 7, 14)  # first chunk written in each bank (zeroes it)
G_STOP = (6, 13, 15)  # last chunk written in each bank


def _build_program(debug=False):
    key = ("nc", debug)
    if key in _PROG_CACHE:
        return _PROG_CACHE[key]

    import concourse.mybir as mybir
    import concourse.tile as tile
    from concourse import bacc

    f32 = mybir.dt.float32
    b16 = mybir.dt.bfloat16
    Exp = mybir.ActivationFunctionType.Exp
    MUL = mybir.AluOpType.mult
    ADD = mybir.AluOpType.add

    nc = bacc.Bacc(None)

    # host layouts are partition-major so each DMA is one large transfer
    x_d = nc.dram_tensor("xT", [128, 8, N], b16, kind="ExternalInput")
    # wqkv columns reordered: block j'=2*hp+t (t=0 q, t=1 k), v at 1024:1536
    wqkv_d = nc.dram_tensor("wqkv", [128, 8, 1536], b16, kind="ExternalInput")
    wp_d = nc.dram_tensor("wp", [128, 4, C], b16, kind="ExternalInput")
    neglam_d = nc.dram_tensor("neglam", [128, 1], f32, kind="ExternalInput")
    ident_d = nc.dram_tensor("ident", [128, 128], b16, kind="ExternalInput")
    out_d = nc.dram_tensor("out", [8, 128, C], b16, kind="ExternalOutput")
    if debug:
        dbg_qkvT = nc.dram_tensor("dbg_qkvT", [128, 8, N], b16, kind="ExternalOutput")
        dbg_vsb = nc.dram_tensor("dbg_vsb", [128, 8, 8, 65], b16, kind="ExternalOutput")
        dbg_oc = nc.dram_tensor("dbg_oc", [128, 8, 4, 2, 64], b16, kind="ExternalOutput")
        dbg_oT = nc.dram_tensor("dbg_oT", [128, 4, N], b16, kind="ExternalOutput")

    with tile.TileContext(nc) as tc:
        with (
            tc.tile_pool(name="io", bufs=1) as iopool,
            tc.tile_pool(name="work", bufs=4) as wpool,
            tc.tile_pool(name="esb", bufs=12) as epool,
            tc.tile_pool(name="ocp", bufs=2) as ocpool,
            tc.tile_pool(name="pS", bufs=2, space="PSUM") as pS,
            tc.tile_pool(name="pO", bufs=1, space="PSUM") as pO,
            tc.tile_pool(name="pP", bufs=1, space="PSUM") as pP,
        ):
            xT = iopool.tile([128, 8, N], b16)
            wqkv = iopool.tile([128, 8, 1536], b16)
            wp = iopool.tile([128, 4, C], b16)
            neglam = iopool.tile([128, 1], f32)
            # qkvT chunk j=2*hp+t: partitions 0-63 even head d0..63,
            # 64-127 odd head d0..63
            qkvT = iopool.tile([128, 8, N], b16)
            # V in [keys, channels] layout; col 64 of each head = ones
            vsb = iopool.tile([128, 8, 8, 65], b16)
            # transposed attention out for proj: [ch-part, hp, n]
            oT = iopool.tile([128, 4, N], b16)
            ident = iopool.tile([128, 128], b16)
            if debug:
                dbg_oc_t = iopool.tile([128, 8, 4, 2, 64], b16)

            warm = iopool.tile([128, 256], b16)
            nc.gpsimd.memset(warm[:], 0.5)
            nc.gpsimd.memset(vsb[:, :, :, 64:65], 1.0)

            # ---------------- DMA in (strict need order) ----------------
            # The shared DMA slot grants waiting transfers in ARBITRARY
            # order, so a late-need transfer that queues early can starve a
            # critical one.  Put everything on the sync queue in exact need
            # order - its ~1.3us per-issue pacing self-throttles - except
            # w[j0,j1] which rides the otherwise-empty scalar queue.
            nc.scalar.dma_start(wqkv[:, :, 0:256], wqkv_d[:, :, 0:256])
            for cc2 in range(4):
                nc.sync.dma_start(
                    xT[:, 2 * cc2 : 2 * cc2 + 2, 0:512],
                    x_d[:, 2 * cc2 : 2 * cc2 + 2, 0:512],
                )
            # v weights before xh1: V-u0 consumes them first (it only needs
            # xh0), while xh1's first consumer is the k2 unit a bit later
            nc.sync.dma_start(wqkv[:, :, 1024:1280], wqkv_d[:, :, 1024:1280])
            nc.sync.dma_start(wqkv[:, :, 1280:1536], wqkv_d[:, :, 1280:1536])
            for h4 in range(2):
                nc.sync.dma_start(
                    xT[:, 4 * h4 : 4 * h4 + 4, 512:1024],
                    x_d[:, 4 * h4 : 4 * h4 + 4, 512:1024],
                )
            nc.gpsimd.dma_start(neglam[:], neglam_d[:])
            for hp in range(1, 4):
                c0 = hp * 256
                nc.sync.dma_start(wqkv[:, :, c0 : c0 + 256], wqkv_d[:, :, c0 : c0 + 256])
            nc.sync.dma_start(wp[:], wp_d[:])
            nc.sync.dma_start(ident[:], ident_d[:])

            # PE warm-up during the input DMAs: the pstate model needs ~3us
            # of continuous execution to reach full clock, so burn it on
            # dummy matmuls into a single scratch PSUM tile (one slot alloc
            # so the pP ring is not serialized)
            wps = pP.tile([128, 512], f32, tag="p", name="warm")
            for w in range(13):
                nc.tensor.matmul(
                    wps[0:1, 0:256], warm[:, 0:1], warm[:], start=True, stop=True
                )

            # ---------------- filler units ------------------------------
            def emit_qkv_unit(hp, t, nh, pool=None):
                # one [128,512] q/k projection chunk -> qkvT[:, 2hp+t, nh]
                j = 2 * hp + t
                pool = pool or pP
                ps = pool.tile([128, 512], f32, tag=pool.name[1].lower(),
                               name=f"qkv{j}_{nh}")
                for cc in range(8):
                    nc.tensor.matmul(
                        ps[:],
                        wqkv[:, cc, j * 128 : (j + 1) * 128],
                        xT[:, cc, nh * 512 : (nh + 1) * 512],
                        start=(cc == 0),
                        stop=(cc == 7),
                    )
                nc.vector.tensor_copy(qkvT[:, j, nh * 512 : (nh + 1) * 512], ps[:])

            qkparts = {}

            def emit_qkv_half(hp, t, nh, part):
                # half-contraction quantum of a q/k unit; halves must be
                # consecutive pP allocations (ring bufs=1)
                j = 2 * hp + t
                if part == 0:
                    qkparts[(j, nh)] = pP.tile([128, 512], f32, tag="p",
                                               name=f"qkv{j}_{nh}")
                ps = qkparts[(j, nh)]
                for cc in range(4 * part, 4 * part + 4):
                    nc.tensor.matmul(
                        ps[:],
                        wqkv[:, cc, j * 128 : (j + 1) * 128],
                        xT[:, cc, nh * 512 : (nh + 1) * 512],
                        start=(cc == 0),
                        stop=(cc == 7),
                    )
                if part == 1:
                    nc.vector.tensor_copy(
                        qkvT[:, j, nh * 512 : (nh + 1) * 512], ps[:]
                    )

            vparts = {}

            def emit_v_half(mc, part):
                # v -> [keys, channels] layout (operands swapped); DVE evac;
                # emitted in two half-contraction quanta to smooth PE load
                if part == 0:
                    vparts[mc] = pP.tile([128, 512], f32, tag="p", name=f"v{mc}")
                ps = vparts[mc]
                for cc in range(4 * part, 4 * part + 4):
                    nc.tensor.matmul(
                        ps[:],
                        xT[:, cc, mc * 128 : (mc + 1) * 128],
                        wqkv[:, cc, 1024:1536],
                        start=(cc == 0),
                        stop=(cc == 7),
                    )
                if part == 1:
                    nc.vector.tensor_copy(
                        vsb[:, mc, :, 0:64], ps.rearrange("p (g d) -> p g d", g=8)
                    )

            def emit_v_unit(mc):
                emit_v_half(mc, 0)
                emit_v_half(mc, 1)

            def emit_proj_unit(ncc, jh, pool, on_act=False):
                # out[ncc n-chunk, jh 512 out-ch] = oT.T @ wp, K=512 (4 ci)
                # tail units evacuate on ACT (idle once the exps are done)
                ps = pool.tile([128, 512], f32, tag=pool.name[1].lower(),
                               name=f"proj{ncc}_{jh}")
                for ci in range(4):
                    nc.tensor.matmul(
                        ps[:],
                        oT[:, ci, ncc * 128 : (ncc + 1) * 128],
                        wp[:, ci, jh * 512 : (jh + 1) * 512],
                        start=(ci == 0),
                        stop=(ci == 3),
                    )
                osb = wpool.tile([128, 512], b16, tag="osb", bufs=6)
                if on_act:
                    # tail: ACT evac + scalar-queue DMA keeps the sync queue
                    # free for the final transposes' waits
                    nc.scalar.copy(osb[:], ps[:])
                    nc.scalar.dma_start(out_d[ncc, :, jh * 512 : (jh + 1) * 512], osb[:])
                else:
                    nc.vector.tensor_copy(osb[:], ps[:])
                    nc.sync.dma_start(out_d[ncc, :, jh * 512 : (jh + 1) * 512], osb[:])

            projA = {}

            def emit_proj_partial(ncc, jh):
                # n-half-1 proj split: accumulate head-pairs 0-2 into PSUM
                # during sweep 6-7 slack, park the partial in SBUF f32
                ps = pP.tile([128, 512], f32, tag="p", name=f"pp{ncc}_{jh}")
                for ci in range(3):
                    nc.tensor.matmul(
                        ps[:],
                        oT[:, ci, ncc * 128 : (ncc + 1) * 128],
                        wp[:, ci, jh * 512 : (jh + 1) * 512],
                        start=(ci == 0),
                        stop=(ci == 2),
                    )
                pa = wpool.tile([128, 512], f32, tag=f"pa{ncc}_{jh}", bufs=1)
                nc.vector.tensor_copy(pa[:], ps[:])
                projA[(ncc, jh)] = pa

            def emit_proj_final(ncc, jh, pool):
                # tail: one head-pair-3 matmul + DVE add of the parked partial
                ps = pool.tile([128, 512], f32, tag=pool.name[1].lower(),
                               name=f"pf{ncc}_{jh}")
                nc.tensor.matmul(
                    ps[:],
                    oT[:, 3, ncc * 128 : (ncc + 1) * 128],
                    wp[:, 3, jh * 512 : (jh + 1) * 512],
                    start=True,
                    stop=True,
                )
                osb = wpool.tile([128, 512], b16, tag="osb", bufs=6)
                nc.vector.tensor_tensor(osb[:], ps[:], projA[(ncc, jh)][:], ADD)
                nc.sync.dma_start(out_d[ncc, :, jh * 512 : (jh + 1) * 512], osb[:])

            # ---------------- attention pieces --------------------------
            def emit_scores_exp(hp, nh, mc):
                # 4 score matmuls + 2 exps; returns e tiles [g0, g1]
                cur = []
                for g in range(2):
                    s_ps = pS.tile([128, 2, 512], f32, tag="s")
                    for i in range(2):
                        rg = RG[2 * g + i]
                        nc.tensor.matmul(
                            s_ps[:, i, :],
                            qkvT[
                                32 * rg : 32 * rg + 32,
                                2 * hp + 1,
                                mc * 128 : (mc + 1) * 128,
                            ],
                            qkvT[
                                32 * rg : 32 * rg + 32,
                                2 * hp,
                                nh * 512 : (nh + 1) * 512,
                            ],
                            start=True,
                            stop=True,
                            tile_position=(32 * rg, 0),
                        )
                    e_sb = epool.tile([128, 2, 512], b16, tag="e")
                    nc.scalar.activation(e_sb[:], s_ps[:], Exp, scale=0.125)
                    cur.append(e_sb)
                return cur

            def emit_pv(hp, etiles, o, mc):
                # stationary-E PV into the 3-bank packed tile o [128,3,512]
                # chunk g=4*ci+nsub at (bank g//7, col (g%7)*65); start=True
                # zeroes the whole bank so only its first chunk may set it
                for ci in range(4):
                    par = ci % 2
                    for nsub in range(4):
                        g = 4 * ci + nsub
                        b, sl = GB[g]
                        nc.tensor.matmul(
                            o[:, b, sl * 65 : sl * 65 + 65],
                            etiles[ci // 2][:, par, nsub * 128 : (nsub + 1) * 128],
                            vsb[:, mc, 2 * hp + par, :],
                            start=(mc == 0 and g in G_START),
                            stop=(mc == 7 and g in G_STOP),
                            skip_group_check=True,
                        )

            def emit_oraw(o):
                # bulk-evacuate the packed accumulator to SBUF (3 copies,
                # one per bank) so the PSUM tile is released fast - the
                # normalization then runs off the critical path from SBUF
                oraw = wpool.tile([128, 16, 65], f32, tag="oraw")
                orv = oraw.rearrange("p g c -> p (g c)")
                for b, cnt in ((0, 7), (1, 7), (2, 2)):
                    nc.vector.tensor_copy(
                        orv[:, b * 455 : b * 455 + cnt * 65], o[:, b, 0 : cnt * 65]
                    )
                return oraw

            def emit_recips(oraw):
                # per-partition reciprocals of the stride-65 denominators
                r = wpool.tile([128, 16, 1], f32, tag="r")
                nc.vector.reciprocal(r[:], oraw[:, :, 64:65])
                # -lam fold on the h2 chunks (g 8..15)
                nc.vector.tensor_scalar_mul(r[:, 8:16], r[:, 8:16], neglam[:])
                return r

            def emit_combine_chunk(hp, oraw, r, oc, nsub):
                # one nsub (128 n cols): 2 parities, fused on DVE, all SBUF
                for par in range(2):
                    g1 = 4 * par + nsub
                    g2 = 8 + 4 * par + nsub
                    t = wpool.tile([128, 64], f32, tag=f"t{par}")
                    nc.vector.tensor_scalar_mul(
                        t[:], oraw[:, g2, 0:64], r[:, g2]
                    )
                    nc.vector.scalar_tensor_tensor(
                        oc[:, nsub, par, :],
                        oraw[:, g1, 0:64],
                        r[:, g1],
                        t[:],
                        MUL,
                        ADD,
                    )

            def emit_combine(hp, nh, o):
                oraw = emit_oraw(o)
                r = emit_recips(oraw)
                oc = ocpool.tile([128, 4, 2, 64], b16, tag="oc")
                for nsub in range(4):
                    emit_combine_chunk(hp, oraw, r, oc, nsub)
                if debug:
                    nc.vector.tensor_copy(dbg_oc_t[:, 2 * hp + nh], oc[:])
                return oc

            def emit_transpose(hp, nh, oc, nsub):
                # oc[128 n, par, 64ch] -> oT[128 ch, n] via DMA xbar
                n0 = nh * 512 + nsub * 128
                nc.sync.dma_start_transpose(oT[:, hp, n0 : n0 + 128], oc[:, nsub])

            def emit_transpose_pe(hp, nh, oc, nsub):
                # final-sweep transpose on PE (PSUM is free, and the ~2.4us
                # DMA-transpose latency would sit on the critical tail)
                n0 = nh * 512 + nsub * 128
                trb = pS.tile([128, 128], b16, tag="s", name="trb")
                nc.tensor.matmul(trb[:], oc[:, nsub], ident[:], is_transpose=True)
                nc.scalar.copy(oT[:, hp, n0 : n0 + 128], trb[:])

            # ---------------- filler schedule ---------------------------
            filler = {}

            def add(slot, fn, *args):
                filler.setdefault(slot, []).append((fn, args))

            # sweep 0 (hp0,nh0): V units + rest of hp0 qkv.  V-u(k) must be
            # emitted by the fillers of slot k+LAG (its PV consumer); the
            # second-half hp1 units are deferred to sweep 2's slack.
            add((0, 0), emit_v_unit, 0)
            add((0, 1), emit_v_unit, 1)
            add((0, 2), emit_qkv_unit, 0, 1, 1)  # k hp0 keys 512-1023 (mc4+)
            add((0, 3), emit_v_unit, 2)
            add((0, 4), emit_v_unit, 3)
            add((0, 5), emit_qkv_unit, 0, 0, 1)  # q hp0 n 512+ (sweep 1)
            add((0, 6), emit_v_unit, 4)
            add((0, 7), emit_v_unit, 5)
            add((1, 0), emit_v_unit, 6)
            add((1, 0), emit_v_unit, 7)
            # sweep 1: first-half hp1 qkv (needed by sweep 2 start),
            # half-quanta at adjacent slots to smooth per-slot PE load
            add((1, 2), emit_qkv_half, 1, 1, 0, 0)
            add((1, 3), emit_qkv_half, 1, 1, 0, 1)
            add((1, 5), emit_qkv_half, 1, 0, 0, 0)
            add((1, 6), emit_qkv_half, 1, 0, 0, 1)
            # sweep 2: rest of hp1 (k-u2 by slot 4, q-u2 by sweep 3)
            add((2, 0), emit_qkv_unit, 1, 1, 1)
            add((2, 2), emit_qkv_unit, 1, 0, 1)
            # sweeps 2-3: hp2 (needed by sweep 4)
            add((2, 5), emit_qkv_unit, 2, 1, 0)
            add((3, 0), emit_qkv_unit, 2, 1, 1)
            add((3, 3), emit_qkv_unit, 2, 0, 0)
            add((3, 6), emit_qkv_unit, 2, 0, 1)
            # sweeps 4-5: hp3 (needed by sweep 6)
            add((4, 2), emit_qkv_unit, 3, 1, 0)
            add((4, 6), emit_qkv_unit, 3, 1, 1)
            add((5, 2), emit_qkv_unit, 3, 0, 0)
            add((5, 6), emit_qkv_unit, 3, 0, 1)
            # sweep 6: three n-half-1 proj partials (hp0-2) at 2-slot
            # spacing so each pP evac completes before the next alloc
            add((6, 3), emit_proj_partial, 4, 0)
            add((6, 4), emit_proj_partial, 4, 1)
            add((6, 5), emit_proj_partial, 5, 0)
            add((6, 6), emit_proj_partial, 5, 1)
            add((6, 7), emit_proj_partial, 6, 0)
            # sweep 7: proj units for n-half 0 ride the per-slot PE slack
            # (evacs on DVE so they don't steal ACT from the exps).  They
            # must come at mc>=2: sweep 6's transposes land in the mc==1
            # combine block, and a unit emitted before them would read
            # oT[:, 3, :] with no registered writer.
            add((7, 2), emit_proj_unit, 0, 0, pP)
            add((7, 3), emit_proj_unit, 0, 1, pP)
            add((7, 4), emit_proj_unit, 1, 0, pP)
            add((7, 5), emit_proj_unit, 1, 1, pP)
            add((7, 6), emit_proj_unit, 2, 0, pP)
            add((7, 7), emit_proj_unit, 2, 1, pP)

            # qkv for the first sweep must precede it.  k and q interleave
            # per 2-cc chunk (k on pP, q on a free pS slot) so both chase
            # the xh0 chunk arrivals; k keys 0-127 evacuate first since the
            # first score matmul only needs that slice.
            kps = pP.tile([128, 512], f32, tag="p", name="k0pre")
            qps = pS.tile([128, 512], f32, tag="s", name="q0pre")
            for cc in range(8):
                nc.tensor.matmul(
                    kps[:], wqkv[:, cc, 128:256], xT[:, cc, 0:512],
                    start=(cc == 0), stop=(cc == 7),
                )
                nc.tensor.matmul(
                    qps[:], wqkv[:, cc, 0:128], xT[:, cc, 0:512],
                    start=(cc == 0), stop=(cc == 7),
                )
            nc.vector.tensor_copy(qkvT[:, 1, 0:128], kps[:, 0:128])
            nc.vector.tensor_copy(qkvT[:, 0, 0:512], qps[:])
            nc.vector.tensor_copy(qkvT[:, 1, 128:512], kps[:, 128:512])

            # ---------------- main pipeline -----------------------------
            # software-pipelined across sweep boundaries: the tail PVs of
            # sweep s are emitted after sweep s+1's first scores/exp so the
            # next exp is never behind them on PE; combine lands at mc==1
            pending = None  # (hp, nh, o_tile)
            tailpv = None  # (hp, edeque, o_tile)
            for s in range(8):
                hp, nh = s // 2, s % 2
                o = None
                edeque = []
                for mc in range(8):
                    edeque.append(emit_scores_exp(hp, nh, mc))
                    for fn, args in filler.get((s, mc), ()):
                        fn(*args)
                    if mc == 0 and tailpv is not None:
                        for mcp in range(8 - LAG, 8):
                            emit_pv(tailpv[0], tailpv[1][mcp], tailpv[2], mcp)
                        tailpv = None
                    if mc == 1 and pending is not None:
                        oc = emit_combine(*pending)
                        for nsub in range(4):
                            emit_transpose(pending[0], pending[1], oc, nsub)
                        pending = None
                    if mc >= LAG:
                        if o is None:
                            o = pO.tile([128, 3, 512], f32, tag="o", name=f"o{s}")
                        emit_pv(hp, edeque[mc - LAG], o, mc - LAG)
                pending = (hp, nh, o)
                tailpv = (hp, edeque, o)

            # final sweep: tail PVs, then chunked combine -> transpose ->
            # proj pipeline; two held-back n-half-0 proj units keep PE warm
            hp, nh, o = pending
            for mcp in range(8 - LAG, 8):
                emit_pv(hp, tailpv[1][mcp], o, mcp)
            oraw = emit_oraw(o)
            r = emit_recips(oraw)
            # held-back n-half-0 proj units fill the combine window on PE;
            # evacs stay on DVE so ACT only carries the tr copies and the
            # two engines pipeline the tail instead of serializing on ACT
            emit_proj_unit(3, 0, pP)
            emit_proj_unit(3, 1, pS)
            oc = ocpool.tile([128, 4, 2, 64], b16, tag="oc")
            for nsub in range(4):
                emit_combine_chunk(hp, oraw, r, oc, nsub)
                emit_transpose_pe(hp, nh, oc, nsub)
                if (4 + nsub, 0) in projA:
                    emit_proj_final(4 + nsub, 0, pP if nsub % 2 else pS)
                else:
                    emit_proj_unit(4 + nsub, 0, pP if nsub % 2 else pS)
                # last group: second evac on the now-idle ACT engine so the
                # two final evac+DMA chains run on parallel engines
                if (4 + nsub, 1) in projA:
                    emit_proj_final(4 + nsub, 1, pS if nsub % 2 else pP)
                else:
                    emit_proj_unit(4 + nsub, 1, pS if nsub % 2 else pP,
                                   on_act=(nsub == 3))
            if debug:
                nc.vector.tensor_copy(dbg_oc_t[:, 2 * hp + nh], oc[:])

            if debug:
                nc.sync.dma_start(dbg_qkvT[:], qkvT[:])
                nc.sync.dma_start(dbg_vsb[:], vsb[:])
                nc.sync.dma_start(dbg_oc[:], dbg_oc_t[:])
                nc.sync.dma_start(dbg_oT[:], oT[:])

    nc.compile()
    _PROG_CACHE[key] = nc
    return nc


def _prep_core_inputs(x, W_qkv, W_proj, neg_lam):
    """Host-side shard prep. Returns in_maps for the 8 cores."""
    W4 = np.asarray(W_qkv, np.float32).reshape(3, H, HD, C)
    in_maps = []
    for core in range(8):
        b, hg = divmod(core, 2)
        xT = (
            np.ascontiguousarray(np.asarray(x[b], np.float32).T)
            .reshape(8, 128, N)
            .transpose(1, 0, 2)
            .astype(BF16)
        )
        wsl = W4[:, hg * 8 : (hg + 1) * 8]  # [3, 8 heads, 64, 1024]
        # columns [t(3), head(8), d(64)]; reorder q/k to j'=2*hp+t blocks
        Wcols = np.ascontiguousarray(wsl.transpose(3, 0, 1, 2).reshape(C, 1536))
        Wnew = np.empty_like(Wcols)
        for hp in range(4):
            for t in range(2):
                src = t * 512 + hp * 128
                dst = (2 * hp + t) * 128
                Wnew[:, dst : dst + 128] = Wcols[:, src : src + 128]
        Wnew[:, 1024:1536] = Wcols[:, 1024:1536]
        wqkv = Wnew.reshape(8, 128, 1536).transpose(1, 0, 2).astype(BF16)
        wp = (
            np.ascontiguousarray(
                np.asarray(W_proj, np.float32)[:, hg * 512 : (hg + 1) * 512].T
            )
            .reshape(4, 128, C)
            .transpose(1, 0, 2)
            .astype(BF16)
        )
        in_maps.append(
            {
                "xT": np.ascontiguousarray(xT),
                "wqkv": np.ascontiguousarray(wqkv),
                "wp": np.ascontiguousarray(wp),
                "neglam": np.full((128, 1), neg_lam, np.float32),
                "ident": np.eye(128, dtype=np.float32).astype(BF16),
            }
        )
    return in_maps


def kernel(x, W_qkv, W_proj, b_proj, lambda_q1, lambda_k1, lambda_q2, lambda_k2):
    from concourse.bass_utils import run_bass_kernel_spmd

    lq1 = np.asarray(lambda_q1, np.float64)
    lk1 = np.asarray(lambda_k1, np.float64)
    lq2 = np.asarray(lambda_q2, np.float64)
    lk2 = np.asarray(lambda_k2, np.float64)
    lam = float(np.mean(np.exp(lq1 * lk1) - np.exp(lq2 * lk2) + LAMBDA_INIT))

    nc = _build_program()
    in_maps = _prep_core_inputs(x, W_qkv, W_proj, -lam)
    res = run_bass_kernel_spmd(nc, in_maps, core_ids=list(range(8)))
    _PROG_CACHE["last_result"] = res

    bp = np.asarray(b_proj, np.float32)
    out = np.empty((B, N, C), np.float32)
    for b in range(B):
        p0 = res.results[2 * b]["out"].astype(np.float32).reshape(N, C)
        p1 = res.results[2 * b + 1]["out"].astype(np.float32).reshape(N, C)
        out[b] = p0 + p1 + bp[None, :]
    return out
